# revision 4
# baseline (speedup 1.0000x reference)
"""Trainium2 Bass kernel for the ModelB graph loss.

Strategy (data-parallel over batch, 8 batches per core):
  * node_masks are contiguous prefix masks; each batch's valid region is the
    top-left [n, n] block of its [N, N] matrices.  The host extracts that
    block (gathering first if a mask is ever non-prefix - sums are node-
    permutation invariant) and packs it into per-(core, slot) padded blocks
    of a shape shared by all 8 cores, so one SPMD program serves all cores.
  * Pad fills are P=0.5, A=0, R=0: every reduced quantity then gets a pad
    contribution of exactly zero except sum(ln(1-P)) and sum(P^2), which are
    (pad_count * ln(.5)) and (pad_count * 0.25) and are subtracted on the
    host (the device's own fp32 ln(.5) is read back from a calibration lane).
  * On device, each slot block lives in SBUF as one [128, T*N] tile; the
    9 masked sums per batch come out as per-partition row-sum columns via
    fused accum_out on ACT/DVE ops.  The host reduces the [128, x] stats
    tensors and finishes the scalar arithmetic in float64.
"""

import sys

for _p in ("/opt/trn_rl_repo", "/root/.axon_site/_ro/trn_rl_repo"):
    if _p not in sys.path:
        sys.path.insert(0, _p)

from contextlib import ExitStack

import numpy as np

import concourse.bass as bass  # noqa: F401  (registers engine methods)
import concourse.tile as tile
from concourse import bacc, mybir
from concourse.bass_utils import run_bass_kernel_spmd

N_CORES = 8
B, N, C = 64, 512, 2
N_SLOTS = B // N_CORES  # 8 batches per core
EPS = 1e-8

# stats_v columns: per slot [DLT, PD, AD, PA, P2, SD2] then coord [MSE, HSQ]
QV = 6
COL_MSE = N_SLOTS * QV
COL_HSQ = COL_MSE + 1
SV_COLS = 64
# stats_a columns: per slot [SA, L1P, ABS]; ln(0.5) calibration at [0, 30]
QA = 3
COL_CAL = 30
SA_COLS = 32

_FT = mybir.dt.float32
_AF = mybir.ActivationFunctionType
_OP = mybir.AluOpType

_build_cache: dict = {}


def _plan_sizes(n_list):
    """Assign batches to (core, slot) and pick shared slot shapes.

    Rank batches by n descending; slot s takes ranks [8s, 8s+8), core c takes
    rank 8s + c.  Slot size Ns = max n in the rank group (same for every
    core), Ts = ceil(Ns / 128).
    """
    order = np.argsort(-np.asarray(n_list), kind="stable")
    sig = []
    assign = {}  # (core, slot) -> batch index
    for s in range(N_SLOTS):
        group = order[s * N_CORES : (s + 1) * N_CORES]
        ns = int(max(n_list[b] for b in group))
        ts = max(1, -(-ns // 128))
        sig.append((ns, ts))
        for c, b in enumerate(group):
            assign[(c, s)] = int(b)
    return tuple(sig), assign


def _build(sig):
    nc = bacc.Bacc("TRN2", target_bir_lowering=False, debug=False,
                   num_devices=N_CORES)

    p_in, a_in, r_in = [], [], []
    for s, (ns, ts) in enumerate(sig):
        p_in.append(nc.dram_tensor(f"p{s}", [ts * 128, ns], _FT,
                                   kind="ExternalInput").ap())
        a_in.append(nc.dram_tensor(f"a{s}", [ts * 128, ns], _FT,
                                   kind="ExternalInput").ap())
        r_in.append(nc.dram_tensor(f"r{s}", [ts * 128, ns], _FT,
                                   kind="ExternalInput").ap())
    pc_in = nc.dram_tensor("pc", [128, 64], _FT, kind="ExternalInput").ap()
    pt_in = nc.dram_tensor("pt", [128, 64], _FT, kind="ExternalInput").ap()
    mc_in = nc.dram_tensor("mc", [128, 64], _FT, kind="ExternalInput").ap()
    cal_in = nc.dram_tensor("cal", [1, 2], _FT, kind="ExternalInput").ap()
    sv_out = nc.dram_tensor("sv", [128, SV_COLS], _FT,
                            kind="ExternalOutput").ap()
    sa_out = nc.dram_tensor("sa", [128, SA_COLS], _FT,
                            kind="ExternalOutput").ap()

    with tile.TileContext(nc) as tc, ExitStack() as ctx:
        pp = ctx.enter_context(tc.tile_pool(name="pp", bufs=2))
        pa = ctx.enter_context(tc.tile_pool(name="pa", bufs=2))
        pr = ctx.enter_context(tc.tile_pool(name="pr", bufs=2))
        pmid = ctx.enter_context(tc.tile_pool(name="pmid", bufs=2))
        pdum = ctx.enter_context(tc.tile_pool(name="pdum", bufs=2))
        pstat = ctx.enter_context(tc.tile_pool(name="pstat", bufs=1))
        psml = ctx.enter_context(tc.tile_pool(name="psml", bufs=1))

        stats_v = pstat.tile([128, SV_COLS], _FT, tag="sv")
        stats_a = pstat.tile([128, SA_COLS], _FT, tag="sa")

        bm05 = pstat.tile([128, 1], _FT, tag="bm05")
        nc.gpsimd.memset(bm05[:], -0.5)
        bm1 = pstat.tile([128, 1], _FT, tag="bm1")
        nc.gpsimd.memset(bm1[:], -1.0)

        def svc(col):
            return stats_v[:, col : col + 1]

        def sac(col):
            return stats_a[:, col : col + 1]

        for s, (ns, ts) in enumerate(sig):
            f = ts * ns
            tp = pp.tile([128, f], _FT, tag="tp")
            nc.sync.dma_start(
                tp[:].rearrange("p (t n) -> p t n", t=ts),
                p_in[s].rearrange("(t p) n -> p t n", p=128))
            ta = pa.tile([128, f], _FT, tag="ta")
            nc.sync.dma_start(
                ta[:].rearrange("p (t n) -> p t n", t=ts),
                a_in[s].rearrange("(t p) n -> p t n", p=128))
            tr = pr.tile([128, f], _FT, tag="tr")
            nc.sync.dma_start(
                tr[:].rearrange("p (t n) -> p t n", t=ts),
                r_in[s].rearrange("(t p) n -> p t n", p=128))

            # ACT engine: logs / abs / copy, with free row-sum accumulators
            lp = pmid.tile([128, f], _FT, tag="lp")
            nc.scalar.activation(lp[:], tp[:], _AF.Ln)
            l1p = pmid.tile([128, f], _FT, tag="l1p")
            nc.scalar.activation(l1p[:], tp[:], _AF.Ln, bias=1.0, scale=-1.0,
                                 accum_out=sac(s * QA + 1))
            da = pdum.tile([128, f], _FT, tag="da")
            nc.scalar.activation(da[:], tp[:], _AF.Abs, bias=bm05[:],
                                 accum_out=sac(s * QA + 2))
            da2 = pdum.tile([128, f], _FT, tag="da")
            nc.scalar.activation(da2[:], ta[:], _AF.Copy,
                                 accum_out=sac(s * QA + 0))

            # DVE: products with fused row-sum accumulators
            dlt = pmid.tile([128, f], _FT, tag="dlt")
            nc.vector.scalar_tensor_tensor(
                dlt[:], lp[:], 1.0, l1p[:], _OP.mult, _OP.subtract,
                accum_out=svc(s * QV + 0))
            dv = pdum.tile([128, f], _FT, tag="dv")
            nc.vector.scalar_tensor_tensor(
                dv[:], tp[:], 1.0, dlt[:], _OP.mult, _OP.mult,
                accum_out=svc(s * QV + 1))
            dv = pdum.tile([128, f], _FT, tag="dv")
            nc.vector.scalar_tensor_tensor(
                dv[:], ta[:], 1.0, dlt[:], _OP.mult, _OP.mult,
                accum_out=svc(s * QV + 2))
            dv = pdum.tile([128, f], _FT, tag="dv")
            nc.vector.scalar_tensor_tensor(
                dv[:], tp[:], 1.0, ta[:], _OP.mult, _OP.mult,
                accum_out=svc(s * QV + 3))
            dv = pdum.tile([128, f], _FT, tag="dv")
            nc.vector.scalar_tensor_tensor(
                dv[:], tp[:], 1.0, tp[:], _OP.mult, _OP.mult,
                accum_out=svc(s * QV + 4))
            sd = pmid.tile([128, f], _FT, tag="sd")
            nc.vector.tensor_sub(sd[:], tr[:], ta[:])
            dv = pdum.tile([128, f], _FT, tag="dv")
            nc.vector.scalar_tensor_tensor(
                dv[:], sd[:], 1.0, sd[:], _OP.mult, _OP.mult,
                accum_out=svc(s * QV + 5))

        # coordinate losses, packed [128, 64] over all 8 local batches
        tpc = psml.tile([128, 64], _FT, tag="tpc")
        nc.sync.dma_start(tpc[:], pc_in[:])
        tpt = psml.tile([128, 64], _FT, tag="tpt")
        nc.sync.dma_start(tpt[:], pt_in[:])
        tmc = psml.tile([128, 64], _FT, tag="tmc")
        nc.sync.dma_start(tmc[:], mc_in[:])
        d = psml.tile([128, 64], _FT, tag="d")
        nc.vector.tensor_sub(d[:], tpc[:], tpt[:])
        dm = psml.tile([128, 64], _FT, tag="dm")
        nc.vector.tensor_mul(dm[:], d[:], tmc[:])
        dsml = psml.tile([128, 64], _FT, tag="dsml")
        nc.vector.scalar_tensor_tensor(
            dsml[:], dm[:], 1.0, dm[:], _OP.mult, _OP.mult,
            accum_out=svc(COL_MSE))
        adm = psml.tile([128, 64], _FT, tag="adm")
        nc.scalar.activation(adm[:], dm[:], _AF.Abs)
        hb = psml.tile([128, 64], _FT, tag="hb")
        nc.scalar.activation(hb[:], adm[:], _AF.Relu, bias=bm1[:])
        dsml2 = psml.tile([128, 64], _FT, tag="dsml2")
        nc.vector.scalar_tensor_tensor(
            dsml2[:], hb[:], 1.0, hb[:], _OP.mult, _OP.mult,
            accum_out=svc(COL_HSQ))

        # calibration: the device's fp32 Ln(0.5)
        tcal = psml.tile([1, 2], _FT, tag="tcal")
        nc.sync.dma_start(tcal[:], cal_in[:])
        nc.scalar.activation(stats_a[0:1, COL_CAL : COL_CAL + 2], tcal[:],
                             _AF.Ln)

        nc.sync.dma_start(sv_out[:], stats_v[:])
        nc.sync.dma_start(sa_out[:], stats_a[:])

    nc.compile()
    return nc


def _huber(x):
    ax = np.abs(x)
    return np.where(ax <= 1.0, 0.5 * x * x, ax - 0.5)


def kernel(predicted_coords, adjacency_matrix, node_counts, raw_similarity,
           temperature, residual_weight, points, adjacency, node_masks,
           _want_results=None):
    masks = np.asarray(node_masks).astype(bool)
    n_list = masks.sum(axis=1).astype(np.int64)
    sig, assign = _plan_sizes(n_list)

    if sig not in _build_cache:
        _build_cache[sig] = _build(sig)
    nc = _build_cache[sig]

    p_full = np.ascontiguousarray(adjacency_matrix, dtype=np.float32)
    a_full = np.ascontiguousarray(adjacency, dtype=np.float32)
    r_full = np.ascontiguousarray(raw_similarity, dtype=np.float32)
    pc_full = np.ascontiguousarray(predicted_coords, dtype=np.float32)
    pt_full = np.ascontiguousarray(points, dtype=np.float32)
    m_f32 = masks.astype(np.float32)

    # valid-node index per batch (prefix fast path; gather fallback)
    valid = []
    for b in range(B):
        n = int(n_list[b])
        if masks[b, :n].all():
            valid.append(None)  # prefix: plain slicing
        else:
            valid.append(np.flatnonzero(masks[b]))

    in_maps = []
    for c in range(N_CORES):
        im = {}
        for s, (ns, ts) in enumerate(sig):
            b = assign[(c, s)]
            n = int(n_list[b])
            bp = np.full((ts * 128, ns), 0.5, np.float32)
            ba = np.zeros((ts * 128, ns), np.float32)
            br = np.zeros((ts * 128, ns), np.float32)
            if valid[b] is None:
                bp[:n, :n] = p_full[b, :n, :n]
                ba[:n, :n] = a_full[b, :n, :n]
                br[:n, :n] = r_full[b, :n, :n]
            else:
                ix = np.ix_(valid[b], valid[b])
                bp[:n, :n] = p_full[b][ix]
                ba[:n, :n] = a_full[b][ix]
                br[:n, :n] = r_full[b][ix]
            im[f"p{s}"] = bp
            im[f"a{s}"] = ba
            im[f"r{s}"] = br
        bs = [assign[(c, s)] for s in range(N_SLOTS)]
        im["pc"] = pc_full[bs].reshape(128, 64)
        im["pt"] = pt_full[bs].reshape(128, 64)
        im["mc"] = np.repeat(m_f32[bs][:, :, None], C, axis=2).reshape(128, 64)
        im["cal"] = np.array([[0.5, 0.5]], np.float32)
        in_maps.append(im)

    res = run_bass_kernel_spmd(nc, in_maps, core_ids=list(range(N_CORES)))
    if _want_results is not None:
        _want_results.append(res)

    # ---- host finalization in float64 ----
    sv = [res.results[c]["sv"].astype(np.float64) for c in range(N_CORES)]
    sa = [res.results[c]["sa"].astype(np.float64) for c in range(N_CORES)]
    lnhalf = float(res.results[0]["sa"][0, COL_CAL])

    n_arr = n_list.astype(np.float64)
    cnt_coord = max(float(n_arr.sum()) * C, 1.0)
    cnt2 = max(float((n_arr ** 2).sum()), 1.0)

    s_mse = sum(float(v[:, COL_MSE].sum()) for v in sv)
    s_hsq = sum(float(v[:, COL_HSQ].sum()) for v in sv)
    coord_mse = s_mse / cnt_coord
    coord_smooth = (0.5 * s_mse - 0.5 * s_hsq) / cnt_coord
    coord_loss = 0.7 * coord_mse + 0.3 * coord_smooth

    edge_sum = 0.0
    sim_sum = 0.0
    ari_loss = 0.0
    conf_pen = 0.0
    for c in range(N_CORES):
        for s, (ns, ts) in enumerate(sig):
            b = assign[(c, s)]
            n = float(n_list[b])
            padcnt = float(ts * 128 * ns) - n * n
            s_a = float(sa[c][:, s * QA + 0].sum())
            s_l1p = float(sa[c][:, s * QA + 1].sum()) - padcnt * lnhalf
            s_abs = float(sa[c][:, s * QA + 2].sum())
            s_dlt = float(sv[c][:, s * QV + 0].sum())
            s_pd = float(sv[c][:, s * QV + 1].sum())
            s_ad = float(sv[c][:, s * QV + 2].sum())
            s_pa = float(sv[c][:, s * QV + 3].sum())
            s_p2 = float(sv[c][:, s * QV + 4].sum()) - padcnt * 0.25
            s_sd2 = float(sv[c][:, s * QV + 5].sum())

            edge_sum += s_l1p + 0.05 * s_dlt + 0.9 * s_ad
            sim_sum += s_sd2

            if 5.0 < n <= 50.0:
                na = np.sqrt(max(s_p2, 0.0))
                nt = np.sqrt(max(s_a, 0.0))
                cos = s_pa / (max(na, EPS) * max(nt, EPS))
                n2 = max(n * n, 1.0)
                ent = -(s_l1p + s_pd) / n2
                contrast = s_abs / n2
                ari_loss += -cos - 0.2 * contrast
                conf_pen += ent

    edge_loss = -edge_sum / cnt2
    similarity_loss = sim_sum / cnt2

    dc = np.asarray(node_counts, np.float64) - n_arr
    count_loss = float(_huber(dc).mean())
    temp_reg = abs(float(temperature) - 1.0)
    res_reg = abs(float(residual_weight) - 0.5)

    total = (1.0 * coord_loss + 2.0 * edge_loss + 0.1 * count_loss
             + 0.3 * similarity_loss + 0.01 * (temp_reg + res_reg)
             + 1.0 * (ari_loss + 0.1 * conf_pen))
    return np.asarray(total, dtype=np.float32)


# revision 12
# speedup vs baseline: 1.2599x; 1.2599x over previous
"""Trainium2 Bass kernel for the ModelB graph loss.

Strategy (data-parallel over batch, 8 batches per core):
  * node_masks are contiguous prefix masks; each batch's valid region is the
    top-left [n, n] block of its [N, N] matrices.  The host extracts that
    block (gathering first if a mask is ever non-prefix - sums are node-
    permutation invariant) and packs it into per-(core, slot) padded blocks
    of a shape shared by all 8 cores, so one SPMD program serves all cores.
  * Pad fills are P=0.5, A=0, R=0: every reduced quantity then gets a pad
    contribution of exactly zero except sum(ln(1-P)) and sum(P^2), which are
    (pad_count * ln(.5)) and (pad_count * 0.25) and are subtracted on the
    host (the device's own fp32 ln(.5) is read back from a calibration lane).
  * On device, each slot block lives in SBUF as one [128, T*N] tile; the
    9 masked sums per batch come out as per-partition row-sum columns via
    fused accum_out on ACT/DVE ops.  The host reduces the [128, x] stats
    tensors and finishes the scalar arithmetic in float64.
"""

import sys

for _p in ("/opt/trn_rl_repo", "/root/.axon_site/_ro/trn_rl_repo"):
    if _p not in sys.path:
        sys.path.insert(0, _p)

from contextlib import ExitStack

import numpy as np

import concourse.bass as bass  # noqa: F401  (registers engine methods)
import concourse.tile as tile
from concourse import bacc, mybir
from concourse.bass_utils import run_bass_kernel_spmd

N_CORES = 8
B, N, C = 64, 512, 2
N_SLOTS = B // N_CORES  # 8 batches per core
EPS = 1e-8

# stats_v columns: per slot [DLT, PD, AD, PA, P2, SD2, ABS]; coord [MSE, HSQ]
QV = 7
COL_MSE = N_SLOTS * QV
COL_HSQ = COL_MSE + 1
SV_COLS = 64
# stats_a columns: per slot [SA, L1P, ABS]; ln(0.5)-accum cal at [0, 30]
QA = 3
COL_CAL = 30
SA_COLS = 32

_FT = mybir.dt.float32
_BF = mybir.dt.bfloat16
_AF = mybir.ActivationFunctionType
_OP = mybir.AluOpType

try:
    import ml_dtypes

    _BF_NP = ml_dtypes.bfloat16
except ImportError:  # pragma: no cover
    _BF_NP = None

_build_cache: dict = {}


def _plan_sizes(n_list):
    """Assign batches to (core, slot) and pick shared slot shapes.

    Rank batches by n descending; slot s takes ranks [8s, 8s+8), core c takes
    rank 8s + c.  Slot size Ns = max n in the rank group (same for every
    core), Ts = ceil(Ns / 128).
    """
    order = np.argsort(-np.asarray(n_list), kind="stable")
    sig = []
    assign = {}  # (core, slot) -> batch index
    for s in range(N_SLOTS):
        group = order[s * N_CORES : (s + 1) * N_CORES]
        ns = int(max(n_list[b] for b in group))
        ts = max(1, -(-ns // 128))
        # ARI-branch quantities are only consumed for batches with n <= 50
        ari = bool(any(n_list[b] <= 50 for b in group))
        sig.append((ns, ts, ari))
        for c, b in enumerate(group):
            assign[(c, s)] = int(b)
    return tuple(sig), assign


def _build(sig):
    nc = bacc.Bacc("TRN2", target_bir_lowering=False, debug=False,
                   num_devices=N_CORES)

    p_in, a_in, r_in = [], [], []
    for s, (ns, ts, ari) in enumerate(sig):
        p_in.append(nc.dram_tensor(f"p{s}", [ts * 128, ns], _BF,
                                   kind="ExternalInput").ap())
        a_in.append(nc.dram_tensor(f"a{s}", [ts * 128, ns], _BF,
                                   kind="ExternalInput").ap())
        r_in.append(nc.dram_tensor(f"r{s}", [ts * 128, ns], _BF,
                                   kind="ExternalInput").ap())
    pc_in = nc.dram_tensor("pc", [128, 64], _FT, kind="ExternalInput").ap()
    pt_in = nc.dram_tensor("pt", [128, 64], _FT, kind="ExternalInput").ap()
    mc_in = nc.dram_tensor("mc", [128, 64], _FT, kind="ExternalInput").ap()
    cal_in = nc.dram_tensor("cal", [1, 2], _BF, kind="ExternalInput").ap()
    sv_out = nc.dram_tensor("sv", [128, SV_COLS], _FT,
                            kind="ExternalOutput").ap()
    sa_out = nc.dram_tensor("sa", [128, SA_COLS], _FT,
                            kind="ExternalOutput").ap()

    with tile.TileContext(nc) as tc, ExitStack() as ctx:
        pp = ctx.enter_context(tc.tile_pool(name="pp", bufs=2))
        pa = ctx.enter_context(tc.tile_pool(name="pa", bufs=2))
        pr = ctx.enter_context(tc.tile_pool(name="pr", bufs=2))
        pmid = ctx.enter_context(tc.tile_pool(name="pmid", bufs=2))
        pdum = ctx.enter_context(tc.tile_pool(name="pdum", bufs=2))
        pstat = ctx.enter_context(tc.tile_pool(name="pstat", bufs=1))
        psml = ctx.enter_context(tc.tile_pool(name="psml", bufs=1))

        stats_v = pstat.tile([128, SV_COLS], _FT, tag="sv")
        stats_a = pstat.tile([128, SA_COLS], _FT, tag="sa")

        bm1 = pstat.tile([128, 1], _FT, tag="bm1")
        nc.gpsimd.memset(bm1[:], -1.0)
        bm05 = pstat.tile([128, 1], _FT, tag="bm05")
        nc.gpsimd.memset(bm05[:], -0.5)

        def svc(col):
            return stats_v[:, col : col + 1]

        def sac(col):
            return stats_a[:, col : col + 1]

        for s, (ns, ts, ari) in enumerate(sig):
            f = ts * ns
            tp = pp.tile([128, f], _BF, tag="tp")
            nc.sync.dma_start(
                tp[:].rearrange("p (t n) -> p t n", t=ts),
                p_in[s].rearrange("(t p) n -> p t n", p=128))
            ta = pa.tile([128, f], _BF, tag="ta")
            nc.sync.dma_start(
                ta[:].rearrange("p (t n) -> p t n", t=ts),
                a_in[s].rearrange("(t p) n -> p t n", p=128))
            tr = pr.tile([128, f], _BF, tag="tr")
            nc.sync.dma_start(
                tr[:].rearrange("p (t n) -> p t n", t=ts),
                r_in[s].rearrange("(t p) n -> p t n", p=128))

            # ACT: the two logs, with the L1P row-sum accumulated for free
            lp = pmid.tile([128, f], _BF, tag="lp")
            nc.scalar.activation(lp[:], tp[:], _AF.Ln)
            l1p = pmid.tile([128, f], _BF, tag="l1p")
            nc.scalar.activation(l1p[:], tp[:], _AF.Ln, bias=1.0, scale=-1.0,
                                 accum_out=sac(s * QA + 1))

            # GPSIMD: the similarity difference (no accum hardware there)
            sd = pmid.tile([128, f], _BF, tag="sd")
            nc.gpsimd.tensor_sub(sd[:], tr[:], ta[:])

            # DVE: bf16 products with fused row-sum accumulators
            dlt = pmid.tile([128, f], _BF, tag="dlt")
            nc.vector.scalar_tensor_tensor(
                dlt[:], lp[:], 1.0, l1p[:], _OP.mult, _OP.subtract,
                accum_out=svc(s * QV + 0))
            dv = pdum.tile([128, f], _BF, tag="dv")
            nc.vector.scalar_tensor_tensor(
                dv[:], ta[:], 1.0, dlt[:], _OP.mult, _OP.mult,
                accum_out=svc(s * QV + 2))
            dv = pdum.tile([128, f], _BF, tag="dv")
            nc.vector.scalar_tensor_tensor(
                dv[:], sd[:], 1.0, sd[:], _OP.mult, _OP.mult,
                accum_out=svc(s * QV + 5))

            if ari:
                # quantities consumed only by the n <= 50 ARI branch
                da2 = pdum.tile([128, f], _BF, tag="da")
                nc.scalar.activation(da2[:], ta[:], _AF.Copy,
                                     accum_out=sac(s * QA + 0))
                da3 = pdum.tile([128, f], _BF, tag="da")
                nc.scalar.activation(da3[:], tp[:], _AF.Abs, bias=bm05[:],
                                     accum_out=sac(s * QA + 2))
                dv = pdum.tile([128, f], _BF, tag="dv")
                nc.vector.scalar_tensor_tensor(
                    dv[:], tp[:], 1.0, dlt[:], _OP.mult, _OP.mult,
                    accum_out=svc(s * QV + 1))
                dv = pdum.tile([128, f], _BF, tag="dv")
                nc.vector.scalar_tensor_tensor(
                    dv[:], tp[:], 1.0, ta[:], _OP.mult, _OP.mult,
                    accum_out=svc(s * QV + 3))
                dv = pdum.tile([128, f], _BF, tag="dv")
                nc.vector.scalar_tensor_tensor(
                    dv[:], tp[:], 1.0, tp[:], _OP.mult, _OP.mult,
                    accum_out=svc(s * QV + 4))

        # coordinate losses, packed [128, 64] over all 8 local batches
        tpc = psml.tile([128, 64], _FT, tag="tpc")
        nc.sync.dma_start(tpc[:], pc_in[:])
        tpt = psml.tile([128, 64], _FT, tag="tpt")
        nc.sync.dma_start(tpt[:], pt_in[:])
        tmc = psml.tile([128, 64], _FT, tag="tmc")
        nc.sync.dma_start(tmc[:], mc_in[:])
        d = psml.tile([128, 64], _FT, tag="d")
        nc.vector.tensor_sub(d[:], tpc[:], tpt[:])
        dm = psml.tile([128, 64], _FT, tag="dm")
        nc.vector.tensor_mul(dm[:], d[:], tmc[:])
        dsml = psml.tile([128, 64], _FT, tag="dsml")
        nc.vector.scalar_tensor_tensor(
            dsml[:], dm[:], 1.0, dm[:], _OP.mult, _OP.mult,
            accum_out=svc(COL_MSE))
        adm = psml.tile([128, 64], _FT, tag="adm")
        nc.scalar.activation(adm[:], dm[:], _AF.Abs)
        hb = psml.tile([128, 64], _FT, tag="hb")
        nc.scalar.activation(hb[:], adm[:], _AF.Relu, bias=bm1[:])
        dsml2 = psml.tile([128, 64], _FT, tag="dsml2")
        nc.vector.scalar_tensor_tensor(
            dsml2[:], hb[:], 1.0, hb[:], _OP.mult, _OP.mult,
            accum_out=svc(COL_HSQ))

        # calibration: mirror the L1P op on pad-valued input; the fp32
        # accumulator then reports exactly 2x the per-element pad term.
        tcal = psml.tile([1, 2], _BF, tag="tcal")
        nc.sync.dma_start(tcal[:], cal_in[:])
        dcal = psml.tile([1, 2], _BF, tag="dcal")
        nc.scalar.activation(dcal[:], tcal[:], _AF.Ln, bias=1.0, scale=-1.0,
                             accum_out=stats_a[0:1, COL_CAL : COL_CAL + 1])

        nc.sync.dma_start(sv_out[:], stats_v[:])
        nc.sync.dma_start(sa_out[:], stats_a[:])

    nc.compile()
    return nc


def _huber(x):
    ax = np.abs(x)
    return np.where(ax <= 1.0, 0.5 * x * x, ax - 0.5)


def kernel(predicted_coords, adjacency_matrix, node_counts, raw_similarity,
           temperature, residual_weight, points, adjacency, node_masks,
           _want_results=None):
    masks = np.asarray(node_masks).astype(bool)
    n_list = masks.sum(axis=1).astype(np.int64)
    sig, assign = _plan_sizes(n_list)

    if sig not in _build_cache:
        _build_cache[sig] = _build(sig)
    nc = _build_cache[sig]

    p_full = np.ascontiguousarray(adjacency_matrix, dtype=np.float32)
    a_full = np.ascontiguousarray(adjacency, dtype=np.float32)
    r_full = np.ascontiguousarray(raw_similarity, dtype=np.float32)
    pc_full = np.ascontiguousarray(predicted_coords, dtype=np.float32)
    pt_full = np.ascontiguousarray(points, dtype=np.float32)
    m_f32 = masks.astype(np.float32)

    # valid-node index per batch (prefix fast path; gather fallback)
    valid = []
    for b in range(B):
        n = int(n_list[b])
        if masks[b, :n].all():
            valid.append(None)  # prefix: plain slicing
        else:
            valid.append(np.flatnonzero(masks[b]))

    in_maps = []
    for c in range(N_CORES):
        im = {}
        for s, (ns, ts, ari) in enumerate(sig):
            b = assign[(c, s)]
            n = int(n_list[b])
            bp = np.full((ts * 128, ns), 0.5, np.float32)
            ba = np.zeros((ts * 128, ns), np.float32)
            br = np.zeros((ts * 128, ns), np.float32)
            if valid[b] is None:
                bp[:n, :n] = p_full[b, :n, :n]
                ba[:n, :n] = a_full[b, :n, :n]
                br[:n, :n] = r_full[b, :n, :n]
            else:
                ix = np.ix_(valid[b], valid[b])
                bp[:n, :n] = p_full[b][ix]
                ba[:n, :n] = a_full[b][ix]
                br[:n, :n] = r_full[b][ix]
            im[f"p{s}"] = bp.astype(_BF_NP)
            im[f"a{s}"] = ba.astype(_BF_NP)
            im[f"r{s}"] = br.astype(_BF_NP)
        bs = [assign[(c, s)] for s in range(N_SLOTS)]
        im["pc"] = pc_full[bs].reshape(128, 64)
        im["pt"] = pt_full[bs].reshape(128, 64)
        im["mc"] = np.repeat(m_f32[bs][:, :, None], C, axis=2).reshape(128, 64)
        im["cal"] = np.array([[0.5, 0.5]], _BF_NP)
        in_maps.append(im)

    res = run_bass_kernel_spmd(nc, in_maps, core_ids=list(range(N_CORES)))
    if _want_results is not None:
        _want_results.append(res)

    # ---- host finalization in float64 ----
    sv = [res.results[c]["sv"].astype(np.float64) for c in range(N_CORES)]
    sa = [res.results[c]["sa"].astype(np.float64) for c in range(N_CORES)]
    lnhalf = float(res.results[0]["sa"][0, COL_CAL]) / 2.0

    n_arr = n_list.astype(np.float64)
    cnt_coord = max(float(n_arr.sum()) * C, 1.0)
    cnt2 = max(float((n_arr ** 2).sum()), 1.0)

    s_mse = sum(float(v[:, COL_MSE].sum()) for v in sv)
    s_hsq = sum(float(v[:, COL_HSQ].sum()) for v in sv)
    coord_mse = s_mse / cnt_coord
    coord_smooth = (0.5 * s_mse - 0.5 * s_hsq) / cnt_coord
    coord_loss = 0.7 * coord_mse + 0.3 * coord_smooth

    edge_sum = 0.0
    sim_sum = 0.0
    ari_loss = 0.0
    conf_pen = 0.0
    for c in range(N_CORES):
        for s, (ns, ts, ari) in enumerate(sig):
            b = assign[(c, s)]
            n = float(n_list[b])
            padcnt = float(ts * 128 * ns) - n * n
            s_l1p = float(sa[c][:, s * QA + 1].sum()) - padcnt * lnhalf
            s_dlt = float(sv[c][:, s * QV + 0].sum())
            s_ad = float(sv[c][:, s * QV + 2].sum())
            s_sd2 = float(sv[c][:, s * QV + 5].sum())

            edge_sum += s_l1p + 0.05 * s_dlt + 0.9 * s_ad
            sim_sum += s_sd2

            if 5.0 < n <= 50.0:
                s_a = float(sa[c][:, s * QA + 0].sum())
                s_abs = float(sa[c][:, s * QA + 2].sum())
                s_pd = float(sv[c][:, s * QV + 1].sum())
                s_pa = float(sv[c][:, s * QV + 3].sum())
                s_p2 = float(sv[c][:, s * QV + 4].sum()) - padcnt * 0.25
                na = np.sqrt(max(s_p2, 0.0))
                nt = np.sqrt(max(s_a, 0.0))
                cos = s_pa / (max(na, EPS) * max(nt, EPS))
                n2 = max(n * n, 1.0)
                ent = -(s_l1p + s_pd) / n2
                contrast = s_abs / n2
                ari_loss += -cos - 0.2 * contrast
                conf_pen += ent

    edge_loss = -edge_sum / cnt2
    similarity_loss = sim_sum / cnt2

    dc = np.asarray(node_counts, np.float64) - n_arr
    count_loss = float(_huber(dc).mean())
    temp_reg = abs(float(temperature) - 1.0)
    res_reg = abs(float(residual_weight) - 0.5)

    total = (1.0 * coord_loss + 2.0 * edge_loss + 0.1 * count_loss
             + 0.3 * similarity_loss + 0.01 * (temp_reg + res_reg)
             + 1.0 * (ari_loss + 0.1 * conf_pen))
    return np.asarray(total, dtype=np.float32)


# revision 13
# speedup vs baseline: 1.2856x; 1.0204x over previous
"""Trainium2 Bass kernel for the ModelB graph loss.

Strategy (data-parallel over batch, 8 batches per core):
  * node_masks are contiguous prefix masks; each batch's valid region is the
    top-left [n, n] block of its [N, N] matrices.  The host extracts that
    block (gathering first if a mask is ever non-prefix - the sums are node-
    permutation invariant) and packs it, padded, into per-(core, slot) blocks
    whose shapes are shared by all 8 cores, so one SPMD program serves all
    cores.  Batches with n <= 50 are packed two per block at partition
    offsets 0/64; their accumulator columns are split by partition range on
    the host.
  * Pad fills are P=0.5, A=0, R=0: every reduced quantity then has a pad
    contribution of exactly zero except sum(ln(1-P)), sum(ln(P)) and
    sum(P^2).  The ln pads are pad_count*ln(.5) (the device's own value is
    read back through a calibration accumulator lane) and cancel exactly in
    sum(DLT) = sum(LP) - sum(L1P); the P^2 pad is pad_count*0.25.
  * On device each block is a [128, T*N] bf16 SBUF tile (DRAM image is
    pre-interleaved so every partition's span is contiguous).  Per-batch
    masked sums come out as per-partition row-sum columns via fused
    accum_out on ACT/DVE ops; products with no accumulator needs run on
    GPSIMD.  ARI-branch quantities are only computed for slots holding
    n <= 50 batches.  The host reduces the [128, x] stats tensors and
    finishes the scalar arithmetic in float64.
"""

import sys

for _p in ("/opt/trn_rl_repo", "/root/.axon_site/_ro/trn_rl_repo"):
    if _p not in sys.path:
        sys.path.insert(0, _p)

from contextlib import ExitStack

import numpy as np

import concourse.bass as bass  # noqa: F401  (registers engine methods)
import concourse.tile as tile
from concourse import bacc, mybir
from concourse.bass_utils import run_bass_kernel_spmd

N_CORES = 8
B, N, C = 64, 512, 2
N_SLOTS = B // N_CORES  # 8 batches per core
EPS = 1e-8

# stats_v per-slot columns: [AD, SD2, PD, PA, P2] (large slots: first 2 only)
QV = 5
# stats_a per-slot columns: [LP, L1P, SA, ABS] (large slots: first 2 only)
QA = 4

_FT = mybir.dt.float32
_BF = mybir.dt.bfloat16
_AF = mybir.ActivationFunctionType
_OP = mybir.AluOpType

try:
    import ml_dtypes

    _BF_NP = ml_dtypes.bfloat16
except ImportError:  # pragma: no cover
    _BF_NP = None

_build_cache: dict = {}


def _plan(n_list):
    """Choose slot shapes shared by all cores and assign batches to them.

    Rank batches by n descending; rank-group s (8 consecutive ranks) gives
    one batch to every core.  Groups whose members all fit in 64 partitions
    are merged pairwise into "P" slots holding two batches per core at
    partition offsets 0/64.

    Returns (sig, assign) where sig is a tuple of slot descriptors
    ("F", ns, ts, ari) or ("P", f, ari), and assign maps
    (core, slot_index, sub) -> batch index.
    """
    n_arr = np.asarray(n_list)
    order = np.argsort(-n_arr, kind="stable")
    groups = []
    for s in range(N_SLOTS):
        g = order[s * N_CORES : (s + 1) * N_CORES]
        groups.append((int(max(n_arr[b] for b in g)), [int(b) for b in g]))

    sig = []
    assign = {}
    slot = 0
    s = 0
    while s < N_SLOTS:
        ns, g = groups[s]
        if s + 1 < N_SLOTS and ns <= 64 and groups[s + 1][0] <= 64:
            ns2, g2 = groups[s + 1]
            f = max(ns, ns2)
            ari = bool(any(n_arr[b] <= 50 for b in g + g2))
            sig.append(("P", f, ari))
            for c in range(N_CORES):
                assign[(c, slot, 0)] = g[c]
                assign[(c, slot, 1)] = g2[c]
            s += 2
        else:
            ts = max(1, -(-ns // 128))
            ari = bool(any(n_arr[b] <= 50 for b in g))
            sig.append(("F", ns, ts, ari))
            for c in range(N_CORES):
                assign[(c, slot, 0)] = g[c]
            s += 1
        slot += 1
    return tuple(sig), assign


def _slot_f(e):
    return e[1] * e[2] if e[0] == "F" else e[1]


def _build(sig):
    nc = bacc.Bacc("TRN2", target_bir_lowering=False, debug=False,
                   num_devices=N_CORES)

    p_in, a_in, r_in = [], [], []
    for s, e in enumerate(sig):
        f = _slot_f(e)
        p_in.append(nc.dram_tensor(f"p{s}", [128, f], _BF,
                                   kind="ExternalInput").ap())
        a_in.append(nc.dram_tensor(f"a{s}", [128, f], _BF,
                                   kind="ExternalInput").ap())
        r_in.append(nc.dram_tensor(f"r{s}", [128, f], _BF,
                                   kind="ExternalInput").ap())
    pc_in = nc.dram_tensor("pc", [128, 64], _FT, kind="ExternalInput").ap()
    pt_in = nc.dram_tensor("pt", [128, 64], _FT, kind="ExternalInput").ap()
    mc_in = nc.dram_tensor("mc", [128, 64], _FT, kind="ExternalInput").ap()
    cal_in = nc.dram_tensor("cal", [1, 2], _BF, kind="ExternalInput").ap()
    nslots = len(sig)
    sv_cols = nslots * QV + 2
    sa_cols = nslots * QA + 1
    sv_out = nc.dram_tensor("sv", [128, sv_cols], _FT,
                            kind="ExternalOutput").ap()
    sa_out = nc.dram_tensor("sa", [128, sa_cols], _FT,
                            kind="ExternalOutput").ap()

    with tile.TileContext(nc) as tc, ExitStack() as ctx:
        pp = ctx.enter_context(tc.tile_pool(name="pp", bufs=3))
        pa = ctx.enter_context(tc.tile_pool(name="pa", bufs=3))
        pr = ctx.enter_context(tc.tile_pool(name="pr", bufs=3))
        pmid = ctx.enter_context(tc.tile_pool(name="pmid", bufs=3))
        pdum = ctx.enter_context(tc.tile_pool(name="pdum", bufs=3))
        pstat = ctx.enter_context(tc.tile_pool(name="pstat", bufs=1))
        psml = ctx.enter_context(tc.tile_pool(name="psml", bufs=1))

        stats_v = pstat.tile([128, sv_cols], _FT, tag="sv")
        stats_a = pstat.tile([128, sa_cols], _FT, tag="sa")
        bm05 = pstat.tile([128, 1], _FT, tag="bm05")
        nc.gpsimd.memset(bm05[:], -0.5)
        bm1 = pstat.tile([128, 1], _FT, tag="bm1")
        nc.gpsimd.memset(bm1[:], -1.0)

        def svc(s, q):
            col = s * QV + q
            return stats_v[:, col : col + 1]

        def sac(s, q):
            col = s * QA + q
            return stats_a[:, col : col + 1]

        for s, e in enumerate(sig):
            f = _slot_f(e)
            ari = e[-1]
            tp = pp.tile([128, f], _BF, tag="tp")
            nc.sync.dma_start(tp[:], p_in[s][:])
            ta = pa.tile([128, f], _BF, tag="ta")
            nc.sync.dma_start(ta[:], a_in[s][:])
            tr = pr.tile([128, f], _BF, tag="tr")
            nc.sync.dma_start(tr[:], r_in[s][:])

            # ACT: the two logs; their accums give S_LP, S_L1P (and so
            # S_DLT = S_LP - S_L1P on the host, ln(.5) pads cancelling)
            lp = pmid.tile([128, f], _BF, tag="lp")
            nc.scalar.activation(lp[:], tp[:], _AF.Ln,
                                 accum_out=sac(s, 0))
            l1p = pmid.tile([128, f], _BF, tag="l1p")
            nc.scalar.activation(l1p[:], tp[:], _AF.Ln, bias=1.0, scale=-1.0,
                                 accum_out=sac(s, 1))

            # GPSIMD: accumulator-free products
            dlt = pmid.tile([128, f], _BF, tag="dlt")
            nc.gpsimd.tensor_sub(dlt[:], lp[:], l1p[:])
            sd = pmid.tile([128, f], _BF, tag="sd")
            nc.gpsimd.tensor_sub(sd[:], tr[:], ta[:])

            # DVE: products whose row sums we need
            dv = pdum.tile([128, f], _BF, tag="dv")
            nc.vector.scalar_tensor_tensor(
                dv[:], ta[:], 1.0, dlt[:], _OP.mult, _OP.mult,
                accum_out=svc(s, 0))
            dv = pdum.tile([128, f], _BF, tag="dv")
            nc.vector.scalar_tensor_tensor(
                dv[:], sd[:], 1.0, sd[:], _OP.mult, _OP.mult,
                accum_out=svc(s, 1))

            if ari:
                # quantities consumed only by the n <= 50 ARI branch
                da2 = pdum.tile([128, f], _BF, tag="da")
                nc.scalar.activation(da2[:], ta[:], _AF.Copy,
                                     accum_out=sac(s, 2))
                da3 = pdum.tile([128, f], _BF, tag="da")
                nc.scalar.activation(da3[:], tp[:], _AF.Abs, bias=bm05[:],
                                     accum_out=sac(s, 3))
                dv = pdum.tile([128, f], _BF, tag="dv")
                nc.vector.scalar_tensor_tensor(
                    dv[:], tp[:], 1.0, dlt[:], _OP.mult, _OP.mult,
                    accum_out=svc(s, 2))
                dv = pdum.tile([128, f], _BF, tag="dv")
                nc.vector.scalar_tensor_tensor(
                    dv[:], tp[:], 1.0, ta[:], _OP.mult, _OP.mult,
                    accum_out=svc(s, 3))
                dv = pdum.tile([128, f], _BF, tag="dv")
                nc.vector.scalar_tensor_tensor(
                    dv[:], tp[:], 1.0, tp[:], _OP.mult, _OP.mult,
                    accum_out=svc(s, 4))

        # coordinate losses, packed [128, 64] over all 8 local batches
        tpc = psml.tile([128, 64], _FT, tag="tpc")
        nc.sync.dma_start(tpc[:], pc_in[:])
        tpt = psml.tile([128, 64], _FT, tag="tpt")
        nc.sync.dma_start(tpt[:], pt_in[:])
        tmc = psml.tile([128, 64], _FT, tag="tmc")
        nc.sync.dma_start(tmc[:], mc_in[:])
        d = psml.tile([128, 64], _FT, tag="d")
        nc.vector.tensor_sub(d[:], tpc[:], tpt[:])
        dm = psml.tile([128, 64], _FT, tag="dm")
        nc.vector.tensor_mul(dm[:], d[:], tmc[:])
        dsml = psml.tile([128, 64], _FT, tag="dsml")
        nc.vector.scalar_tensor_tensor(
            dsml[:], dm[:], 1.0, dm[:], _OP.mult, _OP.mult,
            accum_out=stats_v[:, nslots * QV : nslots * QV + 1])
        adm = psml.tile([128, 64], _FT, tag="adm")
        nc.scalar.activation(adm[:], dm[:], _AF.Abs)
        hb = psml.tile([128, 64], _FT, tag="hb")
        nc.scalar.activation(hb[:], adm[:], _AF.Relu, bias=bm1[:])
        dsml2 = psml.tile([128, 64], _FT, tag="dsml2")
        nc.vector.scalar_tensor_tensor(
            dsml2[:], hb[:], 1.0, hb[:], _OP.mult, _OP.mult,
            accum_out=stats_v[:, nslots * QV + 1 : nslots * QV + 2])

        # calibration: mirror the L1P op on pad-valued input; the fp32
        # accumulator then reports exactly 2x the per-element pad term.
        tcal = psml.tile([1, 2], _BF, tag="tcal")
        nc.sync.dma_start(tcal[:], cal_in[:])
        dcal = psml.tile([1, 2], _BF, tag="dcal")
        nc.scalar.activation(dcal[:], tcal[:], _AF.Ln, bias=1.0, scale=-1.0,
                             accum_out=stats_a[0:1, nslots * QA :
                                               nslots * QA + 1])

        nc.sync.dma_start(sv_out[:], stats_v[:])
        nc.sync.dma_start(sa_out[:], stats_a[:])

    nc.compile()
    return nc


def _huber(x):
    ax = np.abs(x)
    return np.where(ax <= 1.0, 0.5 * x * x, ax - 0.5)


def _interleave(block, ts):
    """[ts*128, n] row-major -> [128, ts*n] with per-partition contiguity."""
    if ts == 1:
        return block
    n = block.shape[1]
    return np.ascontiguousarray(
        block.reshape(ts, 128, n).transpose(1, 0, 2).reshape(128, ts * n))


def kernel(predicted_coords, adjacency_matrix, node_counts, raw_similarity,
           temperature, residual_weight, points, adjacency, node_masks,
           _want_results=None):
    masks = np.asarray(node_masks).astype(bool)
    n_list = masks.sum(axis=1).astype(np.int64)
    sig, assign = _plan(n_list)

    if sig not in _build_cache:
        _build_cache[sig] = _build(sig)
    nc = _build_cache[sig]

    p_full = np.asarray(adjacency_matrix, dtype=np.float32)
    a_full = np.asarray(adjacency, dtype=np.float32)
    r_full = np.asarray(raw_similarity, dtype=np.float32)
    pc_full = np.ascontiguousarray(predicted_coords, dtype=np.float32)
    pt_full = np.ascontiguousarray(points, dtype=np.float32)
    m_f32 = masks.astype(np.float32)

    # valid-node index per batch (prefix fast path; gather fallback)
    valid = []
    for b in range(B):
        n = int(n_list[b])
        if masks[b, :n].all():
            valid.append(None)  # prefix: plain slicing
        else:
            valid.append(np.flatnonzero(masks[b]))

    in_maps = []
    for c in range(N_CORES):
        im = {}
        bs = []
        for s, e in enumerate(sig):
            if e[0] == "F":
                _, ns, ts, ari = e
                b = assign[(c, s, 0)]
                n = int(n_list[b])
                bp = np.full((ts * 128, ns), 0.5, np.float32)
                ba = np.zeros((ts * 128, ns), np.float32)
                br = np.zeros((ts * 128, ns), np.float32)
                if valid[b] is None:
                    bp[:n, :n] = p_full[b, :n, :n]
                    ba[:n, :n] = a_full[b, :n, :n]
                    br[:n, :n] = r_full[b, :n, :n]
                else:
                    ix = np.ix_(valid[b], valid[b])
                    bp[:n, :n] = p_full[b][ix]
                    ba[:n, :n] = a_full[b][ix]
                    br[:n, :n] = r_full[b][ix]
                im[f"p{s}"] = _interleave(bp, ts).astype(_BF_NP)
                im[f"a{s}"] = _interleave(ba, ts).astype(_BF_NP)
                im[f"r{s}"] = _interleave(br, ts).astype(_BF_NP)
                bs.append(b)
            else:
                _, f, ari = e
                bp = np.full((128, f), 0.5, np.float32)
                ba = np.zeros((128, f), np.float32)
                br = np.zeros((128, f), np.float32)
                for sub, off in ((0, 0), (1, 64)):
                    b = assign[(c, s, sub)]
                    n = int(n_list[b])
                    if valid[b] is None:
                        bp[off:off + n, :n] = p_full[b, :n, :n]
                        ba[off:off + n, :n] = a_full[b, :n, :n]
                        br[off:off + n, :n] = r_full[b, :n, :n]
                    else:
                        ix = np.ix_(valid[b], valid[b])
                        bp[off:off + n, :n] = p_full[b][ix]
                        ba[off:off + n, :n] = a_full[b][ix]
                        br[off:off + n, :n] = r_full[b][ix]
                    bs.append(b)
                im[f"p{s}"] = bp.astype(_BF_NP)
                im[f"a{s}"] = ba.astype(_BF_NP)
                im[f"r{s}"] = br.astype(_BF_NP)
        im["pc"] = pc_full[bs].reshape(128, 64)
        im["pt"] = pt_full[bs].reshape(128, 64)
        im["mc"] = np.repeat(m_f32[bs][:, :, None], C, axis=2).reshape(128, 64)
        im["cal"] = np.array([[0.5, 0.5]], _BF_NP)
        in_maps.append(im)

    res = run_bass_kernel_spmd(nc, in_maps, core_ids=list(range(N_CORES)))
    if _want_results is not None:
        _want_results.append(res)

    # ---- host finalization in float64 ----
    nslots = len(sig)
    sv = [res.results[c]["sv"].astype(np.float64) for c in range(N_CORES)]
    sa = [res.results[c]["sa"].astype(np.float64) for c in range(N_CORES)]
    lnhalf = float(res.results[0]["sa"][0, nslots * QA]) / 2.0

    n_arr = n_list.astype(np.float64)
    cnt_coord = max(float(n_arr.sum()) * C, 1.0)
    cnt2 = max(float((n_arr ** 2).sum()), 1.0)

    s_mse = sum(float(v[:, nslots * QV].sum()) for v in sv)
    s_hsq = sum(float(v[:, nslots * QV + 1].sum()) for v in sv)
    coord_mse = s_mse / cnt_coord
    coord_smooth = (0.5 * s_mse - 0.5 * s_hsq) / cnt_coord
    coord_loss = 0.7 * coord_mse + 0.3 * coord_smooth

    edge_sum = 0.0
    sim_sum = 0.0
    ari_loss = 0.0
    conf_pen = 0.0
    for c in range(N_CORES):
        for s, e in enumerate(sig):
            if e[0] == "F":
                _, ns, ts, ari = e
                subs = [(assign[(c, s, 0)], 0, 128, float(ts * 128 * ns))]
            else:
                _, f, ari = e
                subs = [(assign[(c, s, 0)], 0, 64, float(64 * f)),
                        (assign[(c, s, 1)], 64, 128, float(64 * f))]
            for b, lo, hi, area in subs:
                n = float(n_list[b])
                padcnt = area - n * n
                s_lp = float(sa[c][lo:hi, s * QA + 0].sum())
                s_l1p_raw = float(sa[c][lo:hi, s * QA + 1].sum())
                s_l1p = s_l1p_raw - padcnt * lnhalf
                s_dlt = s_lp - s_l1p_raw  # ln(.5) pads cancel exactly
                s_ad = float(sv[c][lo:hi, s * QV + 0].sum())
                s_sd2 = float(sv[c][lo:hi, s * QV + 1].sum())

                edge_sum += s_l1p + 0.05 * s_dlt + 0.9 * s_ad
                sim_sum += s_sd2

                if 5.0 < n <= 50.0:
                    s_a = float(sa[c][lo:hi, s * QA + 2].sum())
                    s_abs = float(sa[c][lo:hi, s * QA + 3].sum())
                    s_pd = float(sv[c][lo:hi, s * QV + 2].sum())
                    s_pa = float(sv[c][lo:hi, s * QV + 3].sum())
                    s_p2 = float(sv[c][lo:hi, s * QV + 4].sum()) \
                        - padcnt * 0.25
                    na = np.sqrt(max(s_p2, 0.0))
                    nt = np.sqrt(max(s_a, 0.0))
                    cos = s_pa / (max(na, EPS) * max(nt, EPS))
                    n2 = max(n * n, 1.0)
                    ent = -(s_l1p + s_pd) / n2
                    contrast = s_abs / n2
                    ari_loss += -cos - 0.2 * contrast
                    conf_pen += ent

    edge_loss = -edge_sum / cnt2
    similarity_loss = sim_sum / cnt2

    dc = np.asarray(node_counts, np.float64) - n_arr
    count_loss = float(_huber(dc).mean())
    temp_reg = abs(float(temperature) - 1.0)
    res_reg = abs(float(residual_weight) - 0.5)

    total = (1.0 * coord_loss + 2.0 * edge_loss + 0.1 * count_loss
             + 0.3 * similarity_loss + 0.01 * (temp_reg + res_reg)
             + 1.0 * (ari_loss + 0.1 * conf_pen))
    return np.asarray(total, dtype=np.float32)


# revision 14
# speedup vs baseline: 1.3858x; 1.0779x over previous
"""Trainium2 Bass kernel for the ModelB graph loss.

Strategy (data-parallel over batch, 8 batches per core):
  * node_masks are contiguous prefix masks; each batch's valid region is the
    top-left [n, n] block of its [N, N] matrices.  The host extracts that
    block (gathering first if a mask is ever non-prefix - the sums are node-
    permutation invariant) and packs it, padded, into per-(core, slot) blocks
    whose shapes are shared by all 8 cores, so one SPMD program serves all
    cores.  Batches with n <= 50 are packed two per block at partition
    offsets 0/64; their accumulator columns are split by partition range on
    the host.
  * Pad fills are P=0.5, A=0, R=0: every reduced quantity then has a pad
    contribution of exactly zero except sum(ln(1-P)), sum(ln(P)) and
    sum(P^2).  The ln pads are pad_count*ln(.5) (the device's own value is
    read back through a calibration accumulator lane) and cancel exactly in
    sum(DLT) = sum(LP) - sum(L1P); the P^2 pad is pad_count*0.25.
  * On device each block is a [128, T*N] bf16 SBUF tile (DRAM image is
    pre-interleaved so every partition's span is contiguous).  Per-batch
    masked sums come out as per-partition row-sum columns via fused
    accum_out on ACT/DVE ops; products with no accumulator needs run on
    GPSIMD.  ARI-branch quantities are only computed for slots holding
    n <= 50 batches.  The host reduces the [128, x] stats tensors and
    finishes the scalar arithmetic in float64.
"""

import sys

for _p in ("/opt/trn_rl_repo", "/root/.axon_site/_ro/trn_rl_repo"):
    if _p not in sys.path:
        sys.path.insert(0, _p)

from contextlib import ExitStack

import numpy as np

import concourse.bass as bass  # noqa: F401  (registers engine methods)
import concourse.tile as tile
from concourse import bacc, mybir
from concourse.bass_utils import run_bass_kernel_spmd

N_CORES = 8
B, N, C = 64, 512, 2
N_SLOTS = B // N_CORES  # 8 batches per core
EPS = 1e-8

# stats_v per-slot columns: [AD, SD2, PD, PA, P2] (large slots: first 2 only)
QV = 5
# stats_a per-slot columns: [LP, L1P, SA, ABS] (large slots: first 2 only)
QA = 4

_FT = mybir.dt.float32
_BF = mybir.dt.bfloat16
_AF = mybir.ActivationFunctionType
_OP = mybir.AluOpType

try:
    import ml_dtypes

    _BF_NP = ml_dtypes.bfloat16
except ImportError:  # pragma: no cover
    _BF_NP = None

_build_cache: dict = {}


def _plan(n_list):
    """Choose slot shapes shared by all cores and assign batches to them.

    Rank batches by n descending; rank-group s (8 consecutive ranks) gives
    one batch to every core.  Groups whose members all fit in 64 partitions
    are merged pairwise into "P" slots holding two batches per core at
    partition offsets 0/64.

    Returns (sig, assign) where sig is a tuple of slot descriptors
    ("F", ns, ts, ari) or ("P", f, ari), and assign maps
    (core, slot_index, sub) -> batch index.
    """
    n_arr = np.asarray(n_list)
    order = np.argsort(-n_arr, kind="stable")
    groups = []
    for s in range(N_SLOTS):
        g = order[s * N_CORES : (s + 1) * N_CORES]
        groups.append((int(max(n_arr[b] for b in g)), [int(b) for b in g]))

    sig = []
    assign = {}
    slot = 0
    s = 0
    while s < N_SLOTS:
        ns, g = groups[s]
        if s + 1 < N_SLOTS and ns <= 64 and groups[s + 1][0] <= 64:
            ns2, g2 = groups[s + 1]
            f = max(ns, ns2)
            ari = bool(any(n_arr[b] <= 50 for b in g + g2))
            sig.append(("P", f, ari))
            for c in range(N_CORES):
                assign[(c, slot, 0)] = g[c]
                assign[(c, slot, 1)] = g2[c]
            s += 2
        else:
            ts = max(1, -(-ns // 128))
            ari = bool(any(n_arr[b] <= 50 for b in g))
            sig.append(("F", ns, ts, ari))
            for c in range(N_CORES):
                assign[(c, slot, 0)] = g[c]
            s += 1
        slot += 1
    return tuple(sig), assign


def _slot_f(e):
    return e[1] * e[2] if e[0] == "F" else e[1]


def _build(sig):
    nc = bacc.Bacc("TRN2", target_bir_lowering=False, debug=False,
                   num_devices=N_CORES)

    p_in, a_in, r_in = [], [], []
    for s, e in enumerate(sig):
        f = _slot_f(e)
        p_in.append(nc.dram_tensor(f"p{s}", [128, f], _BF,
                                   kind="ExternalInput").ap())
        a_in.append(nc.dram_tensor(f"a{s}", [128, f], _BF,
                                   kind="ExternalInput").ap())
        r_in.append(nc.dram_tensor(f"r{s}", [128, f], _BF,
                                   kind="ExternalInput").ap())
    pc_in = nc.dram_tensor("pc", [128, 64], _FT, kind="ExternalInput").ap()
    pt_in = nc.dram_tensor("pt", [128, 64], _FT, kind="ExternalInput").ap()
    mc_in = nc.dram_tensor("mc", [128, 64], _FT, kind="ExternalInput").ap()
    cal_in = nc.dram_tensor("cal", [1, 2], _BF, kind="ExternalInput").ap()
    nslots = len(sig)
    sv_cols = nslots * QV + 2
    sa_cols = nslots * QA + 1
    sv_out = nc.dram_tensor("sv", [128, sv_cols], _FT,
                            kind="ExternalOutput").ap()
    sa_out = nc.dram_tensor("sa", [128, sa_cols], _FT,
                            kind="ExternalOutput").ap()

    with tile.TileContext(nc) as tc, ExitStack() as ctx:
        pp = ctx.enter_context(tc.tile_pool(name="pp", bufs=4))
        pa = ctx.enter_context(tc.tile_pool(name="pa", bufs=4))
        pr = ctx.enter_context(tc.tile_pool(name="pr", bufs=4))
        pmid = ctx.enter_context(tc.tile_pool(name="pmid", bufs=3))
        pdum = ctx.enter_context(tc.tile_pool(name="pdum", bufs=3))
        pstat = ctx.enter_context(tc.tile_pool(name="pstat", bufs=1))
        psml = ctx.enter_context(tc.tile_pool(name="psml", bufs=1))

        stats_v = pstat.tile([128, sv_cols], _FT, tag="sv")
        stats_a = pstat.tile([128, sa_cols], _FT, tag="sa")
        bm05 = pstat.tile([128, 1], _FT, tag="bm05")
        nc.gpsimd.memset(bm05[:], -0.5)
        bm1 = pstat.tile([128, 1], _FT, tag="bm1")
        nc.gpsimd.memset(bm1[:], -1.0)

        def svc(s, q):
            col = s * QV + q
            return stats_v[:, col : col + 1]

        def sac(s, q):
            col = s * QA + q
            return stats_a[:, col : col + 1]

        # coordinate losses, packed [128, 64] over all 8 local batches
        tpc = psml.tile([128, 64], _FT, tag="tpc")
        nc.sync.dma_start(tpc[:], pc_in[:])
        tpt = psml.tile([128, 64], _FT, tag="tpt")
        nc.sync.dma_start(tpt[:], pt_in[:])
        tmc = psml.tile([128, 64], _FT, tag="tmc")
        nc.sync.dma_start(tmc[:], mc_in[:])
        d = psml.tile([128, 64], _FT, tag="d")
        nc.vector.tensor_sub(d[:], tpc[:], tpt[:])
        dm = psml.tile([128, 64], _FT, tag="dm")
        nc.vector.tensor_mul(dm[:], d[:], tmc[:])
        dsml = psml.tile([128, 64], _FT, tag="dsml")
        nc.vector.scalar_tensor_tensor(
            dsml[:], dm[:], 1.0, dm[:], _OP.mult, _OP.mult,
            accum_out=stats_v[:, nslots * QV : nslots * QV + 1])
        adm = psml.tile([128, 64], _FT, tag="adm")
        nc.scalar.activation(adm[:], dm[:], _AF.Abs)
        hb = psml.tile([128, 64], _FT, tag="hb")
        nc.scalar.activation(hb[:], adm[:], _AF.Relu, bias=bm1[:])
        dsml2 = psml.tile([128, 64], _FT, tag="dsml2")
        nc.vector.scalar_tensor_tensor(
            dsml2[:], hb[:], 1.0, hb[:], _OP.mult, _OP.mult,
            accum_out=stats_v[:, nslots * QV + 1 : nslots * QV + 2])

        # calibration: mirror the L1P op on pad-valued input; the fp32
        # accumulator then reports exactly 2x the per-element pad term.
        tcal = psml.tile([1, 2], _BF, tag="tcal")
        nc.sync.dma_start(tcal[:], cal_in[:])
        dcal = psml.tile([1, 2], _BF, tag="dcal")
        nc.scalar.activation(dcal[:], tcal[:], _AF.Ln, bias=1.0, scale=-1.0,
                             accum_out=stats_a[0:1, nslots * QA :
                                               nslots * QA + 1])


        build_order = [s for s, e in enumerate(sig) if e[0] == "P"] + \
            [s for s, e in enumerate(sig) if e[0] == "F"]
        for s in build_order:
            e = sig[s]
            f = _slot_f(e)
            ari = e[-1]
            tp = pp.tile([128, f], _BF, tag="tp")
            nc.sync.dma_start(tp[:], p_in[s][:])
            ta = pa.tile([128, f], _BF, tag="ta")
            nc.sync.dma_start(ta[:], a_in[s][:])
            tr = pr.tile([128, f], _BF, tag="tr")
            nc.sync.dma_start(tr[:], r_in[s][:])

            # ACT: the two logs; their accums give S_LP, S_L1P (and so
            # S_DLT = S_LP - S_L1P on the host, ln(.5) pads cancelling)
            lp = pmid.tile([128, f], _BF, tag="lp")
            nc.scalar.activation(lp[:], tp[:], _AF.Ln,
                                 accum_out=sac(s, 0))
            l1p = pmid.tile([128, f], _BF, tag="l1p")
            nc.scalar.activation(l1p[:], tp[:], _AF.Ln, bias=1.0, scale=-1.0,
                                 accum_out=sac(s, 1))

            # GPSIMD: accumulator-free similarity difference
            sd = pmid.tile([128, f], _BF, tag="sd")
            nc.gpsimd.tensor_sub(sd[:], tr[:], ta[:])

            # DVE tensor_tensor runs bf16 at 2x; no accum needed for DLT
            dlt = pmid.tile([128, f], _BF, tag="dlt")
            nc.vector.tensor_sub(dlt[:], lp[:], l1p[:])

            # DVE: products whose row sums we need
            dv = pdum.tile([128, f], _BF, tag="dv")
            nc.vector.scalar_tensor_tensor(
                dv[:], ta[:], 1.0, dlt[:], _OP.mult, _OP.mult,
                accum_out=svc(s, 0))
            dv = pdum.tile([128, f], _BF, tag="dv")
            nc.vector.scalar_tensor_tensor(
                dv[:], sd[:], 1.0, sd[:], _OP.mult, _OP.mult,
                accum_out=svc(s, 1))

            if ari:
                # quantities consumed only by the n <= 50 ARI branch
                da2 = pdum.tile([128, f], _BF, tag="da")
                nc.scalar.activation(da2[:], ta[:], _AF.Copy,
                                     accum_out=sac(s, 2))
                da3 = pdum.tile([128, f], _BF, tag="da")
                nc.scalar.activation(da3[:], tp[:], _AF.Abs, bias=bm05[:],
                                     accum_out=sac(s, 3))
                dv = pdum.tile([128, f], _BF, tag="dv")
                nc.vector.scalar_tensor_tensor(
                    dv[:], tp[:], 1.0, dlt[:], _OP.mult, _OP.mult,
                    accum_out=svc(s, 2))
                dv = pdum.tile([128, f], _BF, tag="dv")
                nc.vector.scalar_tensor_tensor(
                    dv[:], tp[:], 1.0, ta[:], _OP.mult, _OP.mult,
                    accum_out=svc(s, 3))
                dv = pdum.tile([128, f], _BF, tag="dv")
                nc.vector.scalar_tensor_tensor(
                    dv[:], tp[:], 1.0, tp[:], _OP.mult, _OP.mult,
                    accum_out=svc(s, 4))

        nc.sync.dma_start(sv_out[:], stats_v[:])
        nc.sync.dma_start(sa_out[:], stats_a[:])

    nc.compile()
    return nc


def _huber(x):
    ax = np.abs(x)
    return np.where(ax <= 1.0, 0.5 * x * x, ax - 0.5)


def _interleave(block, ts):
    """[ts*128, n] row-major -> [128, ts*n] with per-partition contiguity."""
    if ts == 1:
        return block
    n = block.shape[1]
    return np.ascontiguousarray(
        block.reshape(ts, 128, n).transpose(1, 0, 2).reshape(128, ts * n))


def kernel(predicted_coords, adjacency_matrix, node_counts, raw_similarity,
           temperature, residual_weight, points, adjacency, node_masks,
           _want_results=None):
    masks = np.asarray(node_masks).astype(bool)
    n_list = masks.sum(axis=1).astype(np.int64)
    sig, assign = _plan(n_list)

    if sig not in _build_cache:
        _build_cache[sig] = _build(sig)
    nc = _build_cache[sig]

    p_full = np.asarray(adjacency_matrix, dtype=np.float32)
    a_full = np.asarray(adjacency, dtype=np.float32)
    r_full = np.asarray(raw_similarity, dtype=np.float32)
    pc_full = np.ascontiguousarray(predicted_coords, dtype=np.float32)
    pt_full = np.ascontiguousarray(points, dtype=np.float32)
    m_f32 = masks.astype(np.float32)

    # valid-node index per batch (prefix fast path; gather fallback)
    valid = []
    for b in range(B):
        n = int(n_list[b])
        if masks[b, :n].all():
            valid.append(None)  # prefix: plain slicing
        else:
            valid.append(np.flatnonzero(masks[b]))

    in_maps = []
    for c in range(N_CORES):
        im = {}
        bs = []
        for s, e in enumerate(sig):
            if e[0] == "F":
                _, ns, ts, ari = e
                b = assign[(c, s, 0)]
                n = int(n_list[b])
                bp = np.full((ts * 128, ns), 0.5, np.float32)
                ba = np.zeros((ts * 128, ns), np.float32)
                br = np.zeros((ts * 128, ns), np.float32)
                if valid[b] is None:
                    bp[:n, :n] = p_full[b, :n, :n]
                    ba[:n, :n] = a_full[b, :n, :n]
                    br[:n, :n] = r_full[b, :n, :n]
                else:
                    ix = np.ix_(valid[b], valid[b])
                    bp[:n, :n] = p_full[b][ix]
                    ba[:n, :n] = a_full[b][ix]
                    br[:n, :n] = r_full[b][ix]
                im[f"p{s}"] = _interleave(bp, ts).astype(_BF_NP)
                im[f"a{s}"] = _interleave(ba, ts).astype(_BF_NP)
                im[f"r{s}"] = _interleave(br, ts).astype(_BF_NP)
                bs.append(b)
            else:
                _, f, ari = e
                bp = np.full((128, f), 0.5, np.float32)
                ba = np.zeros((128, f), np.float32)
                br = np.zeros((128, f), np.float32)
                for sub, off in ((0, 0), (1, 64)):
                    b = assign[(c, s, sub)]
                    n = int(n_list[b])
                    if valid[b] is None:
                        bp[off:off + n, :n] = p_full[b, :n, :n]
                        ba[off:off + n, :n] = a_full[b, :n, :n]
                        br[off:off + n, :n] = r_full[b, :n, :n]
                    else:
                        ix = np.ix_(valid[b], valid[b])
                        bp[off:off + n, :n] = p_full[b][ix]
                        ba[off:off + n, :n] = a_full[b][ix]
                        br[off:off + n, :n] = r_full[b][ix]
                    bs.append(b)
                im[f"p{s}"] = bp.astype(_BF_NP)
                im[f"a{s}"] = ba.astype(_BF_NP)
                im[f"r{s}"] = br.astype(_BF_NP)
        im["pc"] = pc_full[bs].reshape(128, 64)
        im["pt"] = pt_full[bs].reshape(128, 64)
        im["mc"] = np.repeat(m_f32[bs][:, :, None], C, axis=2).reshape(128, 64)
        im["cal"] = np.array([[0.5, 0.5]], _BF_NP)
        in_maps.append(im)

    res = run_bass_kernel_spmd(nc, in_maps, core_ids=list(range(N_CORES)))
    if _want_results is not None:
        _want_results.append(res)

    # ---- host finalization in float64 ----
    nslots = len(sig)
    sv = [res.results[c]["sv"].astype(np.float64) for c in range(N_CORES)]
    sa = [res.results[c]["sa"].astype(np.float64) for c in range(N_CORES)]
    lnhalf = float(res.results[0]["sa"][0, nslots * QA]) / 2.0

    n_arr = n_list.astype(np.float64)
    cnt_coord = max(float(n_arr.sum()) * C, 1.0)
    cnt2 = max(float((n_arr ** 2).sum()), 1.0)

    s_mse = sum(float(v[:, nslots * QV].sum()) for v in sv)
    s_hsq = sum(float(v[:, nslots * QV + 1].sum()) for v in sv)
    coord_mse = s_mse / cnt_coord
    coord_smooth = (0.5 * s_mse - 0.5 * s_hsq) / cnt_coord
    coord_loss = 0.7 * coord_mse + 0.3 * coord_smooth

    edge_sum = 0.0
    sim_sum = 0.0
    ari_loss = 0.0
    conf_pen = 0.0
    for c in range(N_CORES):
        for s, e in enumerate(sig):
            if e[0] == "F":
                _, ns, ts, ari = e
                subs = [(assign[(c, s, 0)], 0, 128, float(ts * 128 * ns))]
            else:
                _, f, ari = e
                subs = [(assign[(c, s, 0)], 0, 64, float(64 * f)),
                        (assign[(c, s, 1)], 64, 128, float(64 * f))]
            for b, lo, hi, area in subs:
                n = float(n_list[b])
                padcnt = area - n * n
                s_lp = float(sa[c][lo:hi, s * QA + 0].sum())
                s_l1p_raw = float(sa[c][lo:hi, s * QA + 1].sum())
                s_l1p = s_l1p_raw - padcnt * lnhalf
                s_dlt = s_lp - s_l1p_raw  # ln(.5) pads cancel exactly
                s_ad = float(sv[c][lo:hi, s * QV + 0].sum())
                s_sd2 = float(sv[c][lo:hi, s * QV + 1].sum())

                edge_sum += s_l1p + 0.05 * s_dlt + 0.9 * s_ad
                sim_sum += s_sd2

                if 5.0 < n <= 50.0:
                    s_a = float(sa[c][lo:hi, s * QA + 2].sum())
                    s_abs = float(sa[c][lo:hi, s * QA + 3].sum())
                    s_pd = float(sv[c][lo:hi, s * QV + 2].sum())
                    s_pa = float(sv[c][lo:hi, s * QV + 3].sum())
                    s_p2 = float(sv[c][lo:hi, s * QV + 4].sum()) \
                        - padcnt * 0.25
                    na = np.sqrt(max(s_p2, 0.0))
                    nt = np.sqrt(max(s_a, 0.0))
                    cos = s_pa / (max(na, EPS) * max(nt, EPS))
                    n2 = max(n * n, 1.0)
                    ent = -(s_l1p + s_pd) / n2
                    contrast = s_abs / n2
                    ari_loss += -cos - 0.2 * contrast
                    conf_pen += ent

    edge_loss = -edge_sum / cnt2
    similarity_loss = sim_sum / cnt2

    dc = np.asarray(node_counts, np.float64) - n_arr
    count_loss = float(_huber(dc).mean())
    temp_reg = abs(float(temperature) - 1.0)
    res_reg = abs(float(residual_weight) - 0.5)

    total = (1.0 * coord_loss + 2.0 * edge_loss + 0.1 * count_loss
             + 0.3 * similarity_loss + 0.01 * (temp_reg + res_reg)
             + 1.0 * (ari_loss + 0.1 * conf_pen))
    return np.asarray(total, dtype=np.float32)


# revision 16
# speedup vs baseline: 1.4803x; 1.0682x over previous
"""Trainium2 Bass kernel for the ModelB graph loss.

Strategy (data-parallel over batch, 8 batches per core):
  * node_masks are contiguous prefix masks; each batch's valid region is the
    top-left [n, n] block of its [N, N] matrices.  The host extracts that
    block (gathering first if a mask is ever non-prefix - the sums are node-
    permutation invariant) and packs it, padded, into per-(core, slot) blocks
    whose shapes are shared by all 8 cores, so one SPMD program serves all
    cores.  Batches with n <= 50 are packed two per block at partition
    offsets 0/64; their accumulator columns are split by partition range on
    the host.
  * Pad fills are P=0.5, A=0, R=0: every reduced quantity then has a pad
    contribution of exactly zero except sum(ln(1-P)), sum(ln(P)) and
    sum(P^2).  The ln pads are pad_count*ln(.5) (the device's own value is
    read back through a calibration accumulator lane) and cancel exactly in
    sum(DLT) = sum(LP) - sum(L1P); the P^2 pad is pad_count*0.25.
  * On device each block is a [128, T*N] bf16 SBUF tile (DRAM image is
    pre-interleaved so every partition's span is contiguous).  Per-batch
    masked sums come out as per-partition row-sum columns via fused
    accum_out on ACT/DVE ops; products with no accumulator needs run on
    GPSIMD.  ARI-branch quantities are only computed for slots holding
    n <= 50 batches.  The host reduces the [128, x] stats tensors and
    finishes the scalar arithmetic in float64.
"""

import sys

for _p in ("/opt/trn_rl_repo", "/root/.axon_site/_ro/trn_rl_repo"):
    if _p not in sys.path:
        sys.path.insert(0, _p)

from contextlib import ExitStack

import numpy as np

import concourse.bass as bass  # noqa: F401  (registers engine methods)
import concourse.tile as tile
from concourse import bacc, mybir
from concourse.bass_utils import run_bass_kernel_spmd

N_CORES = 8
B, N, C = 64, 512, 2
N_SLOTS = B // N_CORES  # 8 batches per core
EPS = 1e-8

# stats_v per-slot columns: [AD, SD2, PD, PA, P2] (large slots: first 2 only)
QV = 5
# stats_a per-slot columns: [LP, L1P, SA, ABS] (large slots: first 2 only)
QA = 4

_FT = mybir.dt.float32
_BF = mybir.dt.bfloat16
_AF = mybir.ActivationFunctionType
_OP = mybir.AluOpType

try:
    import ml_dtypes

    _BF_NP = ml_dtypes.bfloat16
except ImportError:  # pragma: no cover
    _BF_NP = None

_build_cache: dict = {}


def _plan(n_list):
    """Choose slot shapes shared by all cores and assign batches to them.

    Rank batches by n descending; rank-group s (8 consecutive ranks) gives
    one batch to every core.  Groups whose members all fit in 64 partitions
    are merged pairwise into "P" slots holding two batches per core at
    partition offsets 0/64.

    Returns (sig, assign) where sig is a tuple of slot descriptors
    ("F", ns, ts, ari) or ("P", f, ari), and assign maps
    (core, slot_index, sub) -> batch index.
    """
    n_arr = np.asarray(n_list)
    order = np.argsort(-n_arr, kind="stable")
    groups = []
    for s in range(N_SLOTS):
        g = order[s * N_CORES : (s + 1) * N_CORES]
        groups.append((int(max(n_arr[b] for b in g)), [int(b) for b in g]))

    sig = []
    assign = {}
    slot = 0
    s = 0
    while s < N_SLOTS:
        ns, g = groups[s]
        if s + 1 < N_SLOTS and ns <= 64 and groups[s + 1][0] <= 64:
            ns2, g2 = groups[s + 1]
            f = max(ns, ns2)
            ari = bool(any(n_arr[b] <= 50 for b in g + g2))
            sig.append(("P", f, ari))
            for c in range(N_CORES):
                assign[(c, slot, 0)] = g[c]
                assign[(c, slot, 1)] = g2[c]
            s += 2
        else:
            ts = max(1, -(-ns // 128))
            ari = bool(any(n_arr[b] <= 50 for b in g))
            sig.append(("F", ns, ts, ari))
            for c in range(N_CORES):
                assign[(c, slot, 0)] = g[c]
            s += 1
        slot += 1
    return tuple(sig), assign


def _slot_f(e):
    return e[1] * e[2] if e[0] == "F" else e[1]


def _build(sig):
    nc = bacc.Bacc("TRN2", target_bir_lowering=False, debug=False,
                   num_devices=N_CORES)

    p_in, a_in, r_in = [], [], []
    for s, e in enumerate(sig):
        f = _slot_f(e)
        p_in.append(nc.dram_tensor(f"p{s}", [128, f], _BF,
                                   kind="ExternalInput").ap())
        a_in.append(nc.dram_tensor(f"a{s}", [128, f], _BF,
                                   kind="ExternalInput").ap())
        r_in.append(nc.dram_tensor(f"r{s}", [128, f], _BF,
                                   kind="ExternalInput").ap())
    pc_in = nc.dram_tensor("pc", [128, 64], _FT, kind="ExternalInput").ap()
    pt_in = nc.dram_tensor("pt", [128, 64], _FT, kind="ExternalInput").ap()
    mc_in = nc.dram_tensor("mc", [128, 64], _FT, kind="ExternalInput").ap()
    cal_in = nc.dram_tensor("cal", [1, 2], _BF, kind="ExternalInput").ap()
    nslots = len(sig)
    sv_cols = nslots * QV + 2
    sa_cols = nslots * QA + 1
    sv_out = nc.dram_tensor("sv", [128, sv_cols], _FT,
                            kind="ExternalOutput").ap()
    sa_out = nc.dram_tensor("sa", [128, sa_cols], _FT,
                            kind="ExternalOutput").ap()

    with tile.TileContext(nc) as tc, ExitStack() as ctx:
        pp = ctx.enter_context(tc.tile_pool(name="pp", bufs=4))
        pa = ctx.enter_context(tc.tile_pool(name="pa", bufs=4))
        pr = ctx.enter_context(tc.tile_pool(name="pr", bufs=4))
        pmid = ctx.enter_context(tc.tile_pool(name="pmid", bufs=3))
        pdum = ctx.enter_context(tc.tile_pool(name="pdum", bufs=3))
        pstat = ctx.enter_context(tc.tile_pool(name="pstat", bufs=1))
        psml = ctx.enter_context(tc.tile_pool(name="psml", bufs=1))

        stats_v = pstat.tile([128, sv_cols], _FT, tag="sv")
        stats_a = pstat.tile([128, sa_cols], _FT, tag="sa")
        bm05 = pstat.tile([128, 1], _FT, tag="bm05")
        nc.gpsimd.memset(bm05[:], -0.5)
        bm1 = pstat.tile([128, 1], _FT, tag="bm1")
        nc.gpsimd.memset(bm1[:], -1.0)

        def svc(s, q):
            col = s * QV + q
            return stats_v[:, col : col + 1]

        def sac(s, q):
            col = s * QA + q
            return stats_a[:, col : col + 1]

        # coordinate losses, packed [128, 64] over all 8 local batches
        tpc = psml.tile([128, 64], _FT, tag="tpc")
        nc.sync.dma_start(tpc[:], pc_in[:])
        tpt = psml.tile([128, 64], _FT, tag="tpt")
        nc.sync.dma_start(tpt[:], pt_in[:])
        tmc = psml.tile([128, 64], _FT, tag="tmc")
        nc.sync.dma_start(tmc[:], mc_in[:])
        d = psml.tile([128, 64], _FT, tag="d")
        nc.vector.tensor_sub(d[:], tpc[:], tpt[:])
        dm = psml.tile([128, 64], _FT, tag="dm")
        nc.vector.tensor_mul(dm[:], d[:], tmc[:])
        dsml = psml.tile([128, 64], _FT, tag="dsml")
        nc.vector.scalar_tensor_tensor(
            dsml[:], dm[:], 1.0, dm[:], _OP.mult, _OP.mult,
            accum_out=stats_v[:, nslots * QV : nslots * QV + 1])
        adm = psml.tile([128, 64], _FT, tag="adm")
        nc.scalar.activation(adm[:], dm[:], _AF.Abs)
        hb = psml.tile([128, 64], _FT, tag="hb")
        nc.scalar.activation(hb[:], adm[:], _AF.Relu, bias=bm1[:])
        dsml2 = psml.tile([128, 64], _FT, tag="dsml2")
        nc.vector.scalar_tensor_tensor(
            dsml2[:], hb[:], 1.0, hb[:], _OP.mult, _OP.mult,
            accum_out=stats_v[:, nslots * QV + 1 : nslots * QV + 2])

        # calibration: mirror the L1P op on pad-valued input; the fp32
        # accumulator then reports exactly 2x the per-element pad term.
        tcal = psml.tile([1, 2], _BF, tag="tcal")
        nc.sync.dma_start(tcal[:], cal_in[:])
        dcal = psml.tile([1, 2], _BF, tag="dcal")
        nc.scalar.activation(dcal[:], tcal[:], _AF.Ln, bias=1.0, scale=-1.0,
                             accum_out=stats_a[0:1, nslots * QA :
                                               nslots * QA + 1])


        build_order = [s for s, e in enumerate(sig) if e[0] == "P"] + \
            [s for s, e in enumerate(sig) if e[0] == "F"]
        issuers = [nc.sync]
        iss = [0]

        def chunked_load(tile_, src_, f):
            # one dma_start = one HWDGE queue (~21 GB/s); split large loads
            # across partition ranges so they spread over queues/engines
            nchunk = 4 if f >= 1024 else (2 if f >= 300 else 1)
            rows = 128 // nchunk
            for k in range(nchunk):
                eng = issuers[iss[0] % len(issuers)]
                iss[0] += 1
                eng.dma_start(tile_[k * rows:(k + 1) * rows, :],
                              src_[k * rows:(k + 1) * rows, :])

        for s in build_order:
            e = sig[s]
            f = _slot_f(e)
            ari = e[-1]
            tp = pp.tile([128, f], _BF, tag="tp")
            chunked_load(tp, p_in[s], f)
            ta = pa.tile([128, f], _BF, tag="ta")
            chunked_load(ta, a_in[s], f)
            tr = pr.tile([128, f], _BF, tag="tr")
            chunked_load(tr, r_in[s], f)

            # ACT: the two logs; their accums give S_LP, S_L1P (and so
            # S_DLT = S_LP - S_L1P on the host, ln(.5) pads cancelling)
            lp = pmid.tile([128, f], _BF, tag="lp")
            nc.scalar.activation(lp[:], tp[:], _AF.Ln,
                                 accum_out=sac(s, 0))
            l1p = pmid.tile([128, f], _BF, tag="l1p")
            nc.scalar.activation(l1p[:], tp[:], _AF.Ln, bias=1.0, scale=-1.0,
                                 accum_out=sac(s, 1))

            # similarity difference: V tensor_tensor is 2x for bf16; use
            # GPSIMD only for smaller slots to keep V for the big ones
            sd = pmid.tile([128, f], _BF, tag="sd")
            if f >= 1500:
                nc.vector.tensor_sub(sd[:], tr[:], ta[:])
            else:
                nc.gpsimd.tensor_sub(sd[:], tr[:], ta[:])

            # DVE tensor_tensor runs bf16 at 2x; no accum needed for DLT
            dlt = pmid.tile([128, f], _BF, tag="dlt")
            nc.vector.tensor_sub(dlt[:], lp[:], l1p[:])

            # DVE: products whose row sums we need
            dv = pdum.tile([128, f], _BF, tag="dv")
            nc.vector.scalar_tensor_tensor(
                dv[:], ta[:], 1.0, dlt[:], _OP.mult, _OP.mult,
                accum_out=svc(s, 0))
            dv = pdum.tile([128, f], _BF, tag="dv")
            nc.vector.scalar_tensor_tensor(
                dv[:], sd[:], 1.0, sd[:], _OP.mult, _OP.mult,
                accum_out=svc(s, 1))

            if ari:
                # quantities consumed only by the n <= 50 ARI branch
                da2 = pdum.tile([128, f], _BF, tag="da")
                nc.scalar.activation(da2[:], ta[:], _AF.Copy,
                                     accum_out=sac(s, 2))
                da3 = pdum.tile([128, f], _BF, tag="da")
                nc.scalar.activation(da3[:], tp[:], _AF.Abs, bias=bm05[:],
                                     accum_out=sac(s, 3))
                dv = pdum.tile([128, f], _BF, tag="dv")
                nc.vector.scalar_tensor_tensor(
                    dv[:], tp[:], 1.0, dlt[:], _OP.mult, _OP.mult,
                    accum_out=svc(s, 2))
                dv = pdum.tile([128, f], _BF, tag="dv")
                nc.vector.scalar_tensor_tensor(
                    dv[:], tp[:], 1.0, ta[:], _OP.mult, _OP.mult,
                    accum_out=svc(s, 3))
                dv = pdum.tile([128, f], _BF, tag="dv")
                nc.vector.scalar_tensor_tensor(
                    dv[:], tp[:], 1.0, tp[:], _OP.mult, _OP.mult,
                    accum_out=svc(s, 4))

        nc.sync.dma_start(sv_out[:], stats_v[:])
        nc.sync.dma_start(sa_out[:], stats_a[:])

    nc.compile()
    return nc


def _huber(x):
    ax = np.abs(x)
    return np.where(ax <= 1.0, 0.5 * x * x, ax - 0.5)


def _interleave(block, ts):
    """[ts*128, n] row-major -> [128, ts*n] with per-partition contiguity."""
    if ts == 1:
        return block
    n = block.shape[1]
    return np.ascontiguousarray(
        block.reshape(ts, 128, n).transpose(1, 0, 2).reshape(128, ts * n))


def kernel(predicted_coords, adjacency_matrix, node_counts, raw_similarity,
           temperature, residual_weight, points, adjacency, node_masks,
           _want_results=None):
    masks = np.asarray(node_masks).astype(bool)
    n_list = masks.sum(axis=1).astype(np.int64)
    sig, assign = _plan(n_list)

    if sig not in _build_cache:
        _build_cache[sig] = _build(sig)
    nc = _build_cache[sig]

    p_full = np.asarray(adjacency_matrix, dtype=np.float32)
    a_full = np.asarray(adjacency, dtype=np.float32)
    r_full = np.asarray(raw_similarity, dtype=np.float32)
    pc_full = np.ascontiguousarray(predicted_coords, dtype=np.float32)
    pt_full = np.ascontiguousarray(points, dtype=np.float32)
    m_f32 = masks.astype(np.float32)

    # valid-node index per batch (prefix fast path; gather fallback)
    valid = []
    for b in range(B):
        n = int(n_list[b])
        if masks[b, :n].all():
            valid.append(None)  # prefix: plain slicing
        else:
            valid.append(np.flatnonzero(masks[b]))

    in_maps = []
    for c in range(N_CORES):
        im = {}
        bs = []
        for s, e in enumerate(sig):
            if e[0] == "F":
                _, ns, ts, ari = e
                b = assign[(c, s, 0)]
                n = int(n_list[b])
                bp = np.full((ts * 128, ns), 0.5, np.float32)
                ba = np.zeros((ts * 128, ns), np.float32)
                br = np.zeros((ts * 128, ns), np.float32)
                if valid[b] is None:
                    bp[:n, :n] = p_full[b, :n, :n]
                    ba[:n, :n] = a_full[b, :n, :n]
                    br[:n, :n] = r_full[b, :n, :n]
                else:
                    ix = np.ix_(valid[b], valid[b])
                    bp[:n, :n] = p_full[b][ix]
                    ba[:n, :n] = a_full[b][ix]
                    br[:n, :n] = r_full[b][ix]
                im[f"p{s}"] = _interleave(bp, ts).astype(_BF_NP)
                im[f"a{s}"] = _interleave(ba, ts).astype(_BF_NP)
                im[f"r{s}"] = _interleave(br, ts).astype(_BF_NP)
                bs.append(b)
            else:
                _, f, ari = e
                bp = np.full((128, f), 0.5, np.float32)
                ba = np.zeros((128, f), np.float32)
                br = np.zeros((128, f), np.float32)
                for sub, off in ((0, 0), (1, 64)):
                    b = assign[(c, s, sub)]
                    n = int(n_list[b])
                    if valid[b] is None:
                        bp[off:off + n, :n] = p_full[b, :n, :n]
                        ba[off:off + n, :n] = a_full[b, :n, :n]
                        br[off:off + n, :n] = r_full[b, :n, :n]
                    else:
                        ix = np.ix_(valid[b], valid[b])
                        bp[off:off + n, :n] = p_full[b][ix]
                        ba[off:off + n, :n] = a_full[b][ix]
                        br[off:off + n, :n] = r_full[b][ix]
                    bs.append(b)
                im[f"p{s}"] = bp.astype(_BF_NP)
                im[f"a{s}"] = ba.astype(_BF_NP)
                im[f"r{s}"] = br.astype(_BF_NP)
        im["pc"] = pc_full[bs].reshape(128, 64)
        im["pt"] = pt_full[bs].reshape(128, 64)
        im["mc"] = np.repeat(m_f32[bs][:, :, None], C, axis=2).reshape(128, 64)
        im["cal"] = np.array([[0.5, 0.5]], _BF_NP)
        in_maps.append(im)

    res = run_bass_kernel_spmd(nc, in_maps, core_ids=list(range(N_CORES)))
    if _want_results is not None:
        _want_results.append(res)

    # ---- host finalization in float64 ----
    nslots = len(sig)
    sv = [res.results[c]["sv"].astype(np.float64) for c in range(N_CORES)]
    sa = [res.results[c]["sa"].astype(np.float64) for c in range(N_CORES)]
    lnhalf = float(res.results[0]["sa"][0, nslots * QA]) / 2.0

    n_arr = n_list.astype(np.float64)
    cnt_coord = max(float(n_arr.sum()) * C, 1.0)
    cnt2 = max(float((n_arr ** 2).sum()), 1.0)

    s_mse = sum(float(v[:, nslots * QV].sum()) for v in sv)
    s_hsq = sum(float(v[:, nslots * QV + 1].sum()) for v in sv)
    coord_mse = s_mse / cnt_coord
    coord_smooth = (0.5 * s_mse - 0.5 * s_hsq) / cnt_coord
    coord_loss = 0.7 * coord_mse + 0.3 * coord_smooth

    edge_sum = 0.0
    sim_sum = 0.0
    ari_loss = 0.0
    conf_pen = 0.0
    for c in range(N_CORES):
        for s, e in enumerate(sig):
            if e[0] == "F":
                _, ns, ts, ari = e
                subs = [(assign[(c, s, 0)], 0, 128, float(ts * 128 * ns))]
            else:
                _, f, ari = e
                subs = [(assign[(c, s, 0)], 0, 64, float(64 * f)),
                        (assign[(c, s, 1)], 64, 128, float(64 * f))]
            for b, lo, hi, area in subs:
                n = float(n_list[b])
                padcnt = area - n * n
                s_lp = float(sa[c][lo:hi, s * QA + 0].sum())
                s_l1p_raw = float(sa[c][lo:hi, s * QA + 1].sum())
                s_l1p = s_l1p_raw - padcnt * lnhalf
                s_dlt = s_lp - s_l1p_raw  # ln(.5) pads cancel exactly
                s_ad = float(sv[c][lo:hi, s * QV + 0].sum())
                s_sd2 = float(sv[c][lo:hi, s * QV + 1].sum())

                edge_sum += s_l1p + 0.05 * s_dlt + 0.9 * s_ad
                sim_sum += s_sd2

                if 5.0 < n <= 50.0:
                    s_a = float(sa[c][lo:hi, s * QA + 2].sum())
                    s_abs = float(sa[c][lo:hi, s * QA + 3].sum())
                    s_pd = float(sv[c][lo:hi, s * QV + 2].sum())
                    s_pa = float(sv[c][lo:hi, s * QV + 3].sum())
                    s_p2 = float(sv[c][lo:hi, s * QV + 4].sum()) \
                        - padcnt * 0.25
                    na = np.sqrt(max(s_p2, 0.0))
                    nt = np.sqrt(max(s_a, 0.0))
                    cos = s_pa / (max(na, EPS) * max(nt, EPS))
                    n2 = max(n * n, 1.0)
                    ent = -(s_l1p + s_pd) / n2
                    contrast = s_abs / n2
                    ari_loss += -cos - 0.2 * contrast
                    conf_pen += ent

    edge_loss = -edge_sum / cnt2
    similarity_loss = sim_sum / cnt2

    dc = np.asarray(node_counts, np.float64) - n_arr
    count_loss = float(_huber(dc).mean())
    temp_reg = abs(float(temperature) - 1.0)
    res_reg = abs(float(residual_weight) - 0.5)

    total = (1.0 * coord_loss + 2.0 * edge_loss + 0.1 * count_loss
             + 0.3 * similarity_loss + 0.01 * (temp_reg + res_reg)
             + 1.0 * (ari_loss + 0.1 * conf_pen))
    return np.asarray(total, dtype=np.float32)


# revision 17
# speedup vs baseline: 1.5604x; 1.0542x over previous
"""Trainium2 Bass kernel for the ModelB graph loss.

Strategy (data-parallel over batch, 8 batches per core):
  * node_masks are contiguous prefix masks; each batch's valid region is the
    top-left [n, n] block of its [N, N] matrices.  The host extracts that
    block (gathering first if a mask is ever non-prefix - the sums are node-
    permutation invariant) and packs it, padded, into per-(core, slot) blocks
    whose shapes are shared by all 8 cores, so one SPMD program serves all
    cores.  Batches with n <= 50 are packed two per block at partition
    offsets 0/64; their accumulator columns are split by partition range on
    the host.
  * Pad fills are P=0.5, A=0, R=0: every reduced quantity then has a pad
    contribution of exactly zero except sum(ln(1-P)), sum(ln(P)) and
    sum(P^2).  The ln pads are pad_count*ln(.5) (the device's own value is
    read back through a calibration accumulator lane) and cancel exactly in
    sum(DLT) = sum(LP) - sum(L1P); the P^2 pad is pad_count*0.25.
  * On device each block is a [128, T*N] bf16 SBUF tile (DRAM image is
    pre-interleaved so every partition's span is contiguous).  Per-batch
    masked sums come out as per-partition row-sum columns via fused
    accum_out on ACT/DVE ops; products with no accumulator needs run on
    GPSIMD.  ARI-branch quantities are only computed for slots holding
    n <= 50 batches.  The host reduces the [128, x] stats tensors and
    finishes the scalar arithmetic in float64.
"""

import sys

for _p in ("/opt/trn_rl_repo", "/root/.axon_site/_ro/trn_rl_repo"):
    if _p not in sys.path:
        sys.path.insert(0, _p)

from contextlib import ExitStack

import numpy as np

import concourse.bass as bass  # noqa: F401  (registers engine methods)
import concourse.tile as tile
from concourse import bacc, mybir
from concourse.bass_utils import run_bass_kernel_spmd

N_CORES = 8
B, N, C = 64, 512, 2
N_SLOTS = B // N_CORES  # 8 batches per core
EPS = 1e-8

# stats_v per-slot columns: [AD, SD2, PD, PA, P2] (large slots: first 2 only)
QV = 5
# stats_a per-slot columns: [LP, L1P, SA, ABS] (large slots: first 2 only)
QA = 4

_FT = mybir.dt.float32
_BF = mybir.dt.bfloat16
_AF = mybir.ActivationFunctionType
_OP = mybir.AluOpType

try:
    import ml_dtypes

    _BF_NP = ml_dtypes.bfloat16
except ImportError:  # pragma: no cover
    _BF_NP = None

_build_cache: dict = {}


def _plan(n_list):
    """Choose slot shapes shared by all cores and assign batches to them.

    Rank batches by n descending; rank-group s (8 consecutive ranks) gives
    one batch to every core.  Groups whose members all fit in 64 partitions
    are merged pairwise into "P" slots holding two batches per core at
    partition offsets 0/64.

    Returns (sig, assign) where sig is a tuple of slot descriptors
    ("F", ns, ts, ari) or ("P", f, ari), and assign maps
    (core, slot_index, sub) -> batch index.
    """
    n_arr = np.asarray(n_list)
    order = np.argsort(-n_arr, kind="stable")
    groups = []
    for s in range(N_SLOTS):
        g = order[s * N_CORES : (s + 1) * N_CORES]
        groups.append((int(max(n_arr[b] for b in g)), [int(b) for b in g]))

    sig = []
    assign = {}
    slot = 0
    s = 0
    while s < N_SLOTS:
        ns, g = groups[s]
        if s + 1 < N_SLOTS and ns <= 64 and groups[s + 1][0] <= 64:
            ns2, g2 = groups[s + 1]
            f = max(ns, ns2)
            ari = bool(any(n_arr[b] <= 50 for b in g + g2))
            sig.append(("P", f, ari))
            for c in range(N_CORES):
                assign[(c, slot, 0)] = g[c]
                assign[(c, slot, 1)] = g2[c]
            s += 2
        else:
            ts = max(1, -(-ns // 128))
            ari = bool(any(n_arr[b] <= 50 for b in g))
            sig.append(("F", ns, ts, ari))
            for c in range(N_CORES):
                assign[(c, slot, 0)] = g[c]
            s += 1
        slot += 1
    return tuple(sig), assign


def _slot_f(e):
    return e[1] * e[2] if e[0] == "F" else e[1]


def _build(sig):
    nc = bacc.Bacc("TRN2", target_bir_lowering=False, debug=False,
                   num_devices=N_CORES)

    p_in, a_in, r_in = [], [], []
    for s, e in enumerate(sig):
        f = _slot_f(e)
        p_in.append(nc.dram_tensor(f"p{s}", [128, f], _BF,
                                   kind="ExternalInput").ap())
        a_in.append(nc.dram_tensor(f"a{s}", [128, f], _BF,
                                   kind="ExternalInput").ap())
        r_in.append(nc.dram_tensor(f"r{s}", [128, f], _BF,
                                   kind="ExternalInput").ap())
    pc_in = nc.dram_tensor("pc", [128, 64], _FT, kind="ExternalInput").ap()
    pt_in = nc.dram_tensor("pt", [128, 64], _FT, kind="ExternalInput").ap()
    mc_in = nc.dram_tensor("mc", [128, 64], _FT, kind="ExternalInput").ap()
    cal_in = nc.dram_tensor("cal", [1, 2], _BF, kind="ExternalInput").ap()
    nslots = len(sig)
    sv_cols = nslots * QV + 2
    sa_cols = nslots * QA + 1
    sv_out = nc.dram_tensor("sv", [128, sv_cols], _FT,
                            kind="ExternalOutput").ap()
    sa_out = nc.dram_tensor("sa", [128, sa_cols], _FT,
                            kind="ExternalOutput").ap()

    with tile.TileContext(nc) as tc, ExitStack() as ctx:
        pp = ctx.enter_context(tc.tile_pool(name="pp", bufs=4))
        pa = ctx.enter_context(tc.tile_pool(name="pa", bufs=4))
        pr = ctx.enter_context(tc.tile_pool(name="pr", bufs=4))
        pmid = ctx.enter_context(tc.tile_pool(name="pmid", bufs=3))
        pdum = ctx.enter_context(tc.tile_pool(name="pdum", bufs=3))
        pstat = ctx.enter_context(tc.tile_pool(name="pstat", bufs=1))
        psml = ctx.enter_context(tc.tile_pool(name="psml", bufs=1))

        stats_v = pstat.tile([128, sv_cols], _FT, tag="sv")
        stats_a = pstat.tile([128, sa_cols], _FT, tag="sa")
        bm05 = pstat.tile([128, 1], _FT, tag="bm05")
        nc.gpsimd.memset(bm05[:], -0.5)
        bm1 = pstat.tile([128, 1], _FT, tag="bm1")
        nc.gpsimd.memset(bm1[:], -1.0)

        def svc(s, q):
            col = s * QV + q
            return stats_v[:, col : col + 1]

        def sac(s, q):
            col = s * QA + q
            return stats_a[:, col : col + 1]

        # coordinate losses, packed [128, 64] over all 8 local batches
        tpc = psml.tile([128, 64], _FT, tag="tpc")
        nc.sync.dma_start(tpc[:], pc_in[:])
        tpt = psml.tile([128, 64], _FT, tag="tpt")
        nc.sync.dma_start(tpt[:], pt_in[:])
        tmc = psml.tile([128, 64], _FT, tag="tmc")
        nc.sync.dma_start(tmc[:], mc_in[:])
        d = psml.tile([128, 64], _FT, tag="d")
        nc.vector.tensor_sub(d[:], tpc[:], tpt[:])
        dm = psml.tile([128, 64], _FT, tag="dm")
        nc.vector.tensor_mul(dm[:], d[:], tmc[:])
        dsml = psml.tile([128, 64], _FT, tag="dsml")
        nc.vector.scalar_tensor_tensor(
            dsml[:], dm[:], 1.0, dm[:], _OP.mult, _OP.mult,
            accum_out=stats_v[:, nslots * QV : nslots * QV + 1])
        adm = psml.tile([128, 64], _FT, tag="adm")
        nc.scalar.activation(adm[:], dm[:], _AF.Abs)
        hb = psml.tile([128, 64], _FT, tag="hb")
        nc.scalar.activation(hb[:], adm[:], _AF.Relu, bias=bm1[:])
        dsml2 = psml.tile([128, 64], _FT, tag="dsml2")
        nc.vector.scalar_tensor_tensor(
            dsml2[:], hb[:], 1.0, hb[:], _OP.mult, _OP.mult,
            accum_out=stats_v[:, nslots * QV + 1 : nslots * QV + 2])

        # calibration: mirror the L1P op on pad-valued input; the fp32
        # accumulator then reports exactly 2x the per-element pad term.
        tcal = psml.tile([1, 2], _BF, tag="tcal")
        nc.sync.dma_start(tcal[:], cal_in[:])
        dcal = psml.tile([1, 2], _BF, tag="dcal")
        nc.scalar.activation(dcal[:], tcal[:], _AF.Ln, bias=1.0, scale=-1.0,
                             accum_out=stats_a[0:1, nslots * QA :
                                               nslots * QA + 1])


        fslots = [s for s, e in enumerate(sig) if e[0] == "F"]
        pslots = [s for s, e in enumerate(sig) if e[0] == "P"]
        build_order = fslots[:1] + pslots + fslots[1:]
        # dma_starts are completion-serialized on their issuing engine
        # (~600ns+ each); keep them few and split across the two issuers
        issuers = [nc.sync, nc.gpsimd]
        iss = [0]

        def chunked_load(tile_, src_, f):
            eng = issuers[iss[0] % len(issuers)]
            iss[0] += 1
            eng.dma_start(tile_[:], src_[:])

        for s in build_order:
            e = sig[s]
            f = _slot_f(e)
            ari = e[-1]
            tp = pp.tile([128, f], _BF, tag="tp")
            chunked_load(tp, p_in[s], f)
            ta = pa.tile([128, f], _BF, tag="ta")
            chunked_load(ta, a_in[s], f)
            tr = pr.tile([128, f], _BF, tag="tr")
            chunked_load(tr, r_in[s], f)

            # ACT: the two logs; their accums give S_LP, S_L1P (and so
            # S_DLT = S_LP - S_L1P on the host, ln(.5) pads cancelling)
            lp = pmid.tile([128, f], _BF, tag="lp")
            nc.scalar.activation(lp[:], tp[:], _AF.Ln,
                                 accum_out=sac(s, 0))
            l1p = pmid.tile([128, f], _BF, tag="l1p")
            nc.scalar.activation(l1p[:], tp[:], _AF.Ln, bias=1.0, scale=-1.0,
                                 accum_out=sac(s, 1))

            # similarity difference: V tensor_tensor is 2x for bf16; use
            # GPSIMD only for smaller slots to keep V for the big ones
            sd = pmid.tile([128, f], _BF, tag="sd")
            if f >= 1500:
                nc.vector.tensor_sub(sd[:], tr[:], ta[:])
            else:
                nc.gpsimd.tensor_sub(sd[:], tr[:], ta[:])

            # DVE tensor_tensor runs bf16 at 2x; no accum needed for DLT
            dlt = pmid.tile([128, f], _BF, tag="dlt")
            nc.vector.tensor_sub(dlt[:], lp[:], l1p[:])

            # DVE: products whose row sums we need
            dv = pdum.tile([128, f], _BF, tag="dv")
            nc.vector.scalar_tensor_tensor(
                dv[:], ta[:], 1.0, dlt[:], _OP.mult, _OP.mult,
                accum_out=svc(s, 0))
            dv = pdum.tile([128, f], _BF, tag="dv")
            nc.vector.scalar_tensor_tensor(
                dv[:], sd[:], 1.0, sd[:], _OP.mult, _OP.mult,
                accum_out=svc(s, 1))

            if ari:
                # quantities consumed only by the n <= 50 ARI branch
                da2 = pdum.tile([128, f], _BF, tag="da")
                nc.scalar.activation(da2[:], ta[:], _AF.Copy,
                                     accum_out=sac(s, 2))
                da3 = pdum.tile([128, f], _BF, tag="da")
                nc.scalar.activation(da3[:], tp[:], _AF.Abs, bias=bm05[:],
                                     accum_out=sac(s, 3))
                dv = pdum.tile([128, f], _BF, tag="dv")
                nc.vector.scalar_tensor_tensor(
                    dv[:], tp[:], 1.0, dlt[:], _OP.mult, _OP.mult,
                    accum_out=svc(s, 2))
                dv = pdum.tile([128, f], _BF, tag="dv")
                nc.vector.scalar_tensor_tensor(
                    dv[:], tp[:], 1.0, ta[:], _OP.mult, _OP.mult,
                    accum_out=svc(s, 3))
                dv = pdum.tile([128, f], _BF, tag="dv")
                nc.vector.scalar_tensor_tensor(
                    dv[:], tp[:], 1.0, tp[:], _OP.mult, _OP.mult,
                    accum_out=svc(s, 4))

        nc.sync.dma_start(sv_out[:], stats_v[:])
        nc.sync.dma_start(sa_out[:], stats_a[:])

    nc.compile()
    return nc


def _huber(x):
    ax = np.abs(x)
    return np.where(ax <= 1.0, 0.5 * x * x, ax - 0.5)


def _interleave(block, ts):
    """[ts*128, n] row-major -> [128, ts*n] with per-partition contiguity."""
    if ts == 1:
        return block
    n = block.shape[1]
    return np.ascontiguousarray(
        block.reshape(ts, 128, n).transpose(1, 0, 2).reshape(128, ts * n))


def kernel(predicted_coords, adjacency_matrix, node_counts, raw_similarity,
           temperature, residual_weight, points, adjacency, node_masks,
           _want_results=None):
    masks = np.asarray(node_masks).astype(bool)
    n_list = masks.sum(axis=1).astype(np.int64)
    sig, assign = _plan(n_list)

    if sig not in _build_cache:
        _build_cache[sig] = _build(sig)
    nc = _build_cache[sig]

    p_full = np.asarray(adjacency_matrix, dtype=np.float32)
    a_full = np.asarray(adjacency, dtype=np.float32)
    r_full = np.asarray(raw_similarity, dtype=np.float32)
    pc_full = np.ascontiguousarray(predicted_coords, dtype=np.float32)
    pt_full = np.ascontiguousarray(points, dtype=np.float32)
    m_f32 = masks.astype(np.float32)

    # valid-node index per batch (prefix fast path; gather fallback)
    valid = []
    for b in range(B):
        n = int(n_list[b])
        if masks[b, :n].all():
            valid.append(None)  # prefix: plain slicing
        else:
            valid.append(np.flatnonzero(masks[b]))

    in_maps = []
    for c in range(N_CORES):
        im = {}
        bs = []
        for s, e in enumerate(sig):
            if e[0] == "F":
                _, ns, ts, ari = e
                b = assign[(c, s, 0)]
                n = int(n_list[b])
                bp = np.full((ts * 128, ns), 0.5, np.float32)
                ba = np.zeros((ts * 128, ns), np.float32)
                br = np.zeros((ts * 128, ns), np.float32)
                if valid[b] is None:
                    bp[:n, :n] = p_full[b, :n, :n]
                    ba[:n, :n] = a_full[b, :n, :n]
                    br[:n, :n] = r_full[b, :n, :n]
                else:
                    ix = np.ix_(valid[b], valid[b])
                    bp[:n, :n] = p_full[b][ix]
                    ba[:n, :n] = a_full[b][ix]
                    br[:n, :n] = r_full[b][ix]
                im[f"p{s}"] = _interleave(bp, ts).astype(_BF_NP)
                im[f"a{s}"] = _interleave(ba, ts).astype(_BF_NP)
                im[f"r{s}"] = _interleave(br, ts).astype(_BF_NP)
                bs.append(b)
            else:
                _, f, ari = e
                bp = np.full((128, f), 0.5, np.float32)
                ba = np.zeros((128, f), np.float32)
                br = np.zeros((128, f), np.float32)
                for sub, off in ((0, 0), (1, 64)):
                    b = assign[(c, s, sub)]
                    n = int(n_list[b])
                    if valid[b] is None:
                        bp[off:off + n, :n] = p_full[b, :n, :n]
                        ba[off:off + n, :n] = a_full[b, :n, :n]
                        br[off:off + n, :n] = r_full[b, :n, :n]
                    else:
                        ix = np.ix_(valid[b], valid[b])
                        bp[off:off + n, :n] = p_full[b][ix]
                        ba[off:off + n, :n] = a_full[b][ix]
                        br[off:off + n, :n] = r_full[b][ix]
                    bs.append(b)
                im[f"p{s}"] = bp.astype(_BF_NP)
                im[f"a{s}"] = ba.astype(_BF_NP)
                im[f"r{s}"] = br.astype(_BF_NP)
        im["pc"] = pc_full[bs].reshape(128, 64)
        im["pt"] = pt_full[bs].reshape(128, 64)
        im["mc"] = np.repeat(m_f32[bs][:, :, None], C, axis=2).reshape(128, 64)
        im["cal"] = np.array([[0.5, 0.5]], _BF_NP)
        in_maps.append(im)

    res = run_bass_kernel_spmd(nc, in_maps, core_ids=list(range(N_CORES)))
    if _want_results is not None:
        _want_results.append(res)

    # ---- host finalization in float64 ----
    nslots = len(sig)
    sv = [res.results[c]["sv"].astype(np.float64) for c in range(N_CORES)]
    sa = [res.results[c]["sa"].astype(np.float64) for c in range(N_CORES)]
    lnhalf = float(res.results[0]["sa"][0, nslots * QA]) / 2.0

    n_arr = n_list.astype(np.float64)
    cnt_coord = max(float(n_arr.sum()) * C, 1.0)
    cnt2 = max(float((n_arr ** 2).sum()), 1.0)

    s_mse = sum(float(v[:, nslots * QV].sum()) for v in sv)
    s_hsq = sum(float(v[:, nslots * QV + 1].sum()) for v in sv)
    coord_mse = s_mse / cnt_coord
    coord_smooth = (0.5 * s_mse - 0.5 * s_hsq) / cnt_coord
    coord_loss = 0.7 * coord_mse + 0.3 * coord_smooth

    edge_sum = 0.0
    sim_sum = 0.0
    ari_loss = 0.0
    conf_pen = 0.0
    for c in range(N_CORES):
        for s, e in enumerate(sig):
            if e[0] == "F":
                _, ns, ts, ari = e
                subs = [(assign[(c, s, 0)], 0, 128, float(ts * 128 * ns))]
            else:
                _, f, ari = e
                subs = [(assign[(c, s, 0)], 0, 64, float(64 * f)),
                        (assign[(c, s, 1)], 64, 128, float(64 * f))]
            for b, lo, hi, area in subs:
                n = float(n_list[b])
                padcnt = area - n * n
                s_lp = float(sa[c][lo:hi, s * QA + 0].sum())
                s_l1p_raw = float(sa[c][lo:hi, s * QA + 1].sum())
                s_l1p = s_l1p_raw - padcnt * lnhalf
                s_dlt = s_lp - s_l1p_raw  # ln(.5) pads cancel exactly
                s_ad = float(sv[c][lo:hi, s * QV + 0].sum())
                s_sd2 = float(sv[c][lo:hi, s * QV + 1].sum())

                edge_sum += s_l1p + 0.05 * s_dlt + 0.9 * s_ad
                sim_sum += s_sd2

                if 5.0 < n <= 50.0:
                    s_a = float(sa[c][lo:hi, s * QA + 2].sum())
                    s_abs = float(sa[c][lo:hi, s * QA + 3].sum())
                    s_pd = float(sv[c][lo:hi, s * QV + 2].sum())
                    s_pa = float(sv[c][lo:hi, s * QV + 3].sum())
                    s_p2 = float(sv[c][lo:hi, s * QV + 4].sum()) \
                        - padcnt * 0.25
                    na = np.sqrt(max(s_p2, 0.0))
                    nt = np.sqrt(max(s_a, 0.0))
                    cos = s_pa / (max(na, EPS) * max(nt, EPS))
                    n2 = max(n * n, 1.0)
                    ent = -(s_l1p + s_pd) / n2
                    contrast = s_abs / n2
                    ari_loss += -cos - 0.2 * contrast
                    conf_pen += ent

    edge_loss = -edge_sum / cnt2
    similarity_loss = sim_sum / cnt2

    dc = np.asarray(node_counts, np.float64) - n_arr
    count_loss = float(_huber(dc).mean())
    temp_reg = abs(float(temperature) - 1.0)
    res_reg = abs(float(residual_weight) - 0.5)

    total = (1.0 * coord_loss + 2.0 * edge_loss + 0.1 * count_loss
             + 0.3 * similarity_loss + 0.01 * (temp_reg + res_reg)
             + 1.0 * (ari_loss + 0.1 * conf_pen))
    return np.asarray(total, dtype=np.float32)
